# revision 6
# baseline (speedup 1.0000x reference)
"""DMV inside algorithm (Eisner chart DP, logsumexp semiring) on Trainium2.

Strategy (v2)
-------------
Pure data parallelism: 4096 sentences -> 8 cores x 512; per core ONE pass of
[128 SBUF partitions] x [G=4 sentence groups], all tables bf16 so every big
DVE tensor_tensor runs in 2x_1p mode (0.52 ns/elem).

Exp-domain DP with a positive-only boundary decomposition (no cancellation,
bf16-safe). Tables per group, diag-packed (row = span width, col = start):
  KR/KL: complete-without-stop, row 0 == 1
  SIR[r,c] = eIR[r,c] * srh[c+r]  (right-incomplete, child's R-stop folded)
  SIL[r,c] = eIL[r,c] * slh[c]    (left-incomplete,  child's L-stop folded)
Recurrences per width w (s = 41-w):
  S[i]   = sum_{t=1..w-2} KR[t,i] * KL[w-1-t, i+1+t]          (shared!)
  SIR[w] = PA1*S + PA0*KL[w-1,i+1] + PA3*KR[w-1,i]
  SIL[w] = PB1*S + PB0*KL[w-1,i+1] + PB3*KR[w-1,i]
  KR[w]  = sum_{t=0..w-2} SIR[t+1,i]*KR[w-1-t,i+1+t] + SIR[w,i]*rhoR[i+w]
  KL[w]  = sum_{t=0..w-2} KL[t+1,i]*SIL[w-1-t,i+1+t] + SIL[w,i]*rhoL[i]
with all six coefficient tables and rhoR=srn/srh, rhoL=sln/slh positive,
precomputed on host. Reductions are in-place binary trees of bf16 adds.
Renorm at w=20 rescales row d by an exact power of two 2^(-k*d) (exponent
trick), k returned per sentence (dsum) and undone on host.

Host covers len<=6 sentences with an exact f64 mini-DP (the 2e-2 relative
gate implies tiny absolute budgets only for very short sentences).
"""

import os

os.environ.setdefault("JAX_PLATFORMS", "cpu")

import numpy as np
import ml_dtypes

import concourse.bass as bass  # noqa: F401  (registers engine classes)
import concourse.tile as tile
import bass_rust
from concourse import bacc, mybir

F32 = mybir.dt.float32
BF16 = mybir.dt.bfloat16
AF = mybir.ActivationFunctionType
OP = mybir.AluOpType
AX = mybir.AxisListType
BFNP = ml_dtypes.bfloat16

N = 41
D = 1681            # table pitch N*N
G = 4               # sentence groups per partition (1 pass = 512/core)
NCORES = 8
B_CORE = 128 * G
NTAB = 6            # coef tables: PA1, PB1, PA0, PB0, PA3, PB3
COEF_IN = NTAB * D
STOP_IN = 2 * N     # rhoR, rhoL
RENORM_W = 20
SHORT_LEN = 6       # host computes len <= SHORT_LEN exactly

# banks slots: 0..3 KR g0..3 | 4..7 SIL | 8..11 SIR | 12..15 KL
S_KR, S_SIL, S_SIR, S_KL = 0, 4, 8, 12

# bf16 scratch (element offsets)
PAP = 384           # per-group pitch of A-side product buffer (max (w-2)*s)
PBP = 424           # per-q pitch of B-side product buffer (max w*s)
ZB_PA = 0
ZB_PB = ZB_PA + 4 * PAP          # 1536
ZB_U0 = ZB_PB + 8 * PBP          # 4928
ZB_U3 = ZB_U0 + 2 * G * N
ZB_U03 = ZB_U3 + 2 * G * N
ZB_U1 = ZB_U03 + 2 * G * N
ZB_MB = ZB_U1 + 2 * G * N
ZB_TOTAL = ZB_MB + G * 42

# f32 scratch
ZF_MU2 = 0          # [2,4]
ZF_MU = 8           # [4]
ZF_LM = 12          # [4]
ZF_SC = 16          # [4] 127-k
ZF_SCI = 20         # [4] bit-built 2^-k
ZF_DSUM = 24        # [4]
ZF_M = 28           # [4,42]
ZF_OUT = ZF_M + G * 42           # [4,40]
ZF_TOTAL = ZF_OUT + G * 40

LN2_32 = 32.0 * float(np.log(2.0))


def ap_of(t, offset, dims, lead=None):
    ap = t.copy()
    first = list(t.ap[0]) if lead is None else list(lead)
    ap.ap = bass_rust.VecI64Pair([first] + [list(d) for d in dims])
    ap.offset = offset
    return ap


def build_nc():
    nc = bacc.Bacc("TRN2", target_bir_lowering=False, debug=False, num_devices=1)
    coefs_in = nc.dram_tensor("coefs", [B_CORE, COEF_IN], BF16, kind="ExternalInput").ap()
    stops_in = nc.dram_tensor("stops", [B_CORE, STOP_IN], BF16, kind="ExternalInput").ap()
    logs_d = nc.dram_tensor("ecr", [B_CORE, N - 1], F32, kind="ExternalOutput").ap()
    dsum_d = nc.dram_tensor("dsum", [B_CORE], F32, kind="ExternalOutput").ap()

    with tile.TileContext(nc) as tc:
        with tc.tile_pool(name="p", bufs=1) as pool:
            banks_t = pool.tile([128, 16 * D], BF16)
            coefs_t = pool.tile([128, G * COEF_IN], BF16)
            stops_t = pool.tile([128, G * STOP_IN], BF16)
            zb_t = pool.tile([128, ZB_TOTAL], BF16)
            zf_t = pool.tile([128, ZF_TOTAL], F32)
            banks = banks_t[:]
            coefs = coefs_t[:]
            stops = stops_t[:]
            zb = zb_t[:]
            zf = zf_t[:]
            zi = zf.bitcast(mybir.dt.int32)

            v = nc.vector
            sc = nc.scalar

            # ---- input DMA ----
            # stops: dest [vec][g][41] <- dram row p*G+g cols vec*41+i
            for vec in range(2):
                nc.sync.dma_start(
                    ap_of(stops, vec * G * N, [[N, G], [1, N]]),
                    ap_of(stops_in, vec * N, [[STOP_IN, G], [1, N]],
                          lead=[G * STOP_IN, 128]),
                )
            # coefs: per group, row-chunked so later rows stream under the DP
            for lo, hi in ((0, 3 * N), (3 * N, 9 * N), (9 * N, 21 * N),
                           (21 * N, D)):
                for g in range(G):
                    nc.sync.dma_start(
                        ap_of(coefs, g * COEF_IN + lo, [[D, NTAB], [1, hi - lo]]),
                        ap_of(coefs_in, g * COEF_IN + lo, [[D, NTAB], [1, hi - lo]],
                              lead=[G * COEF_IN, 128]),
                    )

            # ---- init ----
            # zero rows 0..RENORM_W of all 16 slots (renorm reads full rows)
            nc.gpsimd.memset(
                ap_of(banks, 0, [[D, 16], [1, (RENORM_W + 1) * N]]), 0.0)
            v.memset(ap_of(zf, ZF_DSUM, [[1, G]]), 0.0)
            # KR[0,:] = KL[0,:] = 1
            v.memset(ap_of(banks, S_KR * D, [[12 * D, 2], [D, G], [1, N]]), 1.0)

            # ---- chart DP ----
            for w in range(1, N):
                s = N - w
                # A-side shared interior product + tree reduce
                if w >= 3:
                    na = w - 2
                    v.tensor_tensor(
                        ap_of(zb, ZB_PA, [[PAP, G], [s, na], [1, s]]),
                        ap_of(banks, S_KR * D + N, [[D, G], [N, na], [1, s]]),
                        ap_of(banks, S_KL * D + (w - 2) * N + 2,
                              [[D, G], [-(N - 1), na], [1, s]]),
                        OP.mult,
                    )
                    T = na
                    while T > 1:
                        h = T // 2
                        v.tensor_tensor(
                            ap_of(zb, ZB_PA, [[PAP, G], [s, h], [1, s]]),
                            ap_of(zb, ZB_PA, [[PAP, G], [s, h], [1, s]]),
                            ap_of(zb, ZB_PA + (T - h) * s,
                                  [[PAP, G], [s, h], [1, s]]),
                            OP.add,
                        )
                        T -= h
                # u0 = {PA0,PB0}[w] * KL[w-1, i+1]
                v.tensor_tensor(
                    ap_of(zb, ZB_U0, [[G * N, 2], [N, G], [1, s]]),
                    ap_of(banks, S_KL * D + (w - 1) * N + 1,
                          [[0, 2], [D, G], [1, s]]),
                    ap_of(coefs, 2 * D + w * N, [[D, 2], [COEF_IN, G], [1, s]]),
                    OP.mult,
                )
                # u3 = {PA3,PB3}[w] * KR[w-1, i]
                v.tensor_tensor(
                    ap_of(zb, ZB_U3, [[G * N, 2], [N, G], [1, s]]),
                    ap_of(banks, S_KR * D + (w - 1) * N,
                          [[0, 2], [D, G], [1, s]]),
                    ap_of(coefs, 4 * D + w * N, [[D, 2], [COEF_IN, G], [1, s]]),
                    OP.mult,
                )
                wr_ap = ap_of(banks, S_SIR * D + w * N,
                              [[-4 * D, 2], [D, G], [1, s]])
                if w >= 3:
                    v.tensor_tensor(
                        ap_of(zb, ZB_U03, [[G * N, 2], [N, G], [1, s]]),
                        ap_of(zb, ZB_U0, [[G * N, 2], [N, G], [1, s]]),
                        ap_of(zb, ZB_U3, [[G * N, 2], [N, G], [1, s]]),
                        OP.add,
                    )
                    # u1 = {PA1,PB1}[w] * S
                    v.tensor_tensor(
                        ap_of(zb, ZB_U1, [[G * N, 2], [N, G], [1, s]]),
                        ap_of(zb, ZB_PA, [[0, 2], [PAP, G], [1, s]]),
                        ap_of(coefs, 0 * D + w * N, [[D, 2], [COEF_IN, G], [1, s]]),
                        OP.mult,
                    )
                    # SIR[w]/SIL[w] = u03 + u1
                    v.tensor_tensor(
                        wr_ap,
                        ap_of(zb, ZB_U03, [[G * N, 2], [N, G], [1, s]]),
                        ap_of(zb, ZB_U1, [[G * N, 2], [N, G], [1, s]]),
                        OP.add,
                    )
                else:
                    # SIR[w]/SIL[w] = u0 + u3 (no interior)
                    v.tensor_tensor(
                        wr_ap,
                        ap_of(zb, ZB_U0, [[G * N, 2], [N, G], [1, s]]),
                        ap_of(zb, ZB_U3, [[G * N, 2], [N, G], [1, s]]),
                        OP.add,
                    )
                # B side: correction slab (t = w-1) then interior products
                kout_ap = ap_of(banks, S_KR * D + w * N,
                                [[12 * D, 2], [D, G], [1, s]])
                corr_out = (
                    kout_ap if w == 1 else
                    ap_of(zb, ZB_PB + (w - 1) * s, [[4 * PBP, 2], [PBP, G], [1, s]])
                )
                v.tensor_tensor(
                    corr_out,
                    ap_of(banks, S_SIR * D + w * N, [[-4 * D, 2], [D, G], [1, s]]),
                    ap_of(stops, w, [[G * N - w, 2], [N, G], [1, s]]),
                    OP.mult,
                )
                if w >= 2:
                    nb = w - 1
                    v.tensor_tensor(
                        ap_of(zb, ZB_PB, [[PBP, 8], [s, nb], [1, s]]),
                        ap_of(banks, S_SIR * D + N, [[D, 8], [N, nb], [1, s]]),
                        ap_of(banks, (w - 1) * N + 1, [[D, 8], [-(N - 1), nb], [1, s]]),
                        OP.mult,
                    )
                    T = w  # nb interior slabs + corr slab
                    while T > 2:
                        h = T // 2
                        v.tensor_tensor(
                            ap_of(zb, ZB_PB, [[PBP, 8], [s, h], [1, s]]),
                            ap_of(zb, ZB_PB, [[PBP, 8], [s, h], [1, s]]),
                            ap_of(zb, ZB_PB + (T - h) * s, [[PBP, 8], [s, h], [1, s]]),
                            OP.add,
                        )
                        T -= h
                    v.tensor_tensor(
                        kout_ap,
                        ap_of(zb, ZB_PB, [[4 * PBP, 2], [PBP, G], [1, s]]),
                        ap_of(zb, ZB_PB + s, [[4 * PBP, 2], [PBP, G], [1, s]]),
                        OP.add,
                    )

                if w == RENORM_W:
                    s0 = N - w
                    # mu[g] = max over KR/KL row w
                    v.reduce_max(
                        ap_of(zf, ZF_MU2, [[G, 2], [1, G]]),
                        ap_of(banks, S_KR * D + w * N, [[12 * D, 2], [D, G], [1, s0]]),
                        axis=AX.X,
                    )
                    v.tensor_tensor(
                        ap_of(zf, ZF_MU, [[1, G]]),
                        ap_of(zf, ZF_MU2, [[1, G]]),
                        ap_of(zf, ZF_MU2 + G, [[1, G]]),
                        OP.max,
                    )
                    v.tensor_scalar_mul(
                        ap_of(zf, ZF_MU, [[1, G]]), ap_of(zf, ZF_MU, [[1, G]]),
                        2.0 ** -32)
                    v.tensor_scalar_max(
                        ap_of(zf, ZF_MU, [[1, G]]), ap_of(zf, ZF_MU, [[1, G]]),
                        1e-36)
                    sc.activation(
                        ap_of(zf, ZF_LM, [[1, G]]), ap_of(zf, ZF_MU, [[1, G]]),
                        AF.Ln)
                    # k = round((ln(mu*2^-32) + 32 ln2) / (w ln2)); exact int
                    v.tensor_scalar(
                        ap_of(zf, ZF_LM, [[1, G]]), ap_of(zf, ZF_LM, [[1, G]]),
                        LN2_32, 1.0 / (w * float(np.log(2.0))),
                        OP.add, OP.mult,
                    )
                    v.tensor_scalar(
                        ap_of(zf, ZF_LM, [[1, G]]), ap_of(zf, ZF_LM, [[1, G]]),
                        12582912.0, 12582912.0, OP.add, OP.subtract,
                    )
                    v.tensor_tensor(
                        ap_of(zf, ZF_DSUM, [[1, G]]),
                        ap_of(zf, ZF_DSUM, [[1, G]]),
                        ap_of(zf, ZF_LM, [[1, G]]),
                        OP.add,
                    )
                    # 2^-k via exponent bits: (127 - k) << 23
                    v.tensor_scalar(
                        ap_of(zf, ZF_SC, [[1, G]]), ap_of(zf, ZF_LM, [[1, G]]),
                        -1.0, 127.0, OP.mult, OP.add,
                    )
                    v.tensor_copy(
                        ap_of(zi, ZF_SCI, [[1, G]]), ap_of(zf, ZF_SC, [[1, G]]))
                    v.tensor_scalar(
                        ap_of(zi, ZF_SCI, [[1, G]]), ap_of(zi, ZF_SCI, [[1, G]]),
                        23, None, OP.arith_shift_left,
                    )
                    # M[g,d] = 2^(-k d), multiplicative scan; then bf16 copy
                    v.memset(ap_of(zf, ZF_M, [[42, G], [1, 1]]), 1.0)
                    for g in range(G):
                        sca = ap_of(zf, ZF_SCI + g, [[0, N]])
                        v.tensor_tensor_scan(
                            ap_of(zf, ZF_M + g * 42 + 1, [[1, N]]),
                            sca, sca, 1.0, OP.mult, OP.bypass,
                        )
                    v.tensor_copy(
                        ap_of(zb, ZB_MB, [[1, G * 42]]),
                        ap_of(zf, ZF_M, [[1, G * 42]]))
                    # rescale tables rows 0..w (row d scaled by 2^(-k d))
                    for blk in range(4):
                        tap = ap_of(banks, blk * 4 * D,
                                    [[D, G], [N, w + 1], [1, N]])
                        v.tensor_tensor(
                            tap, tap,
                            ap_of(zb, ZB_MB, [[42, G], [1, w + 1], [0, N]]),
                            OP.mult,
                        )
                    # coef rows w+1..40: one extra arc factor 2^-k
                    cap = ap_of(coefs, (w + 1) * N,
                                [[COEF_IN, G], [D, NTAB], [1, (N - 1 - w) * N]])
                    v.tensor_tensor(
                        cap, cap,
                        ap_of(zb, ZB_MB + 1, [[42, G], [0, NTAB], [0, (N - 1 - w) * N]]),
                        OP.mult,
                    )

            # ---- output: KR col 0 rows 1..40 (raw exp domain) + dsum ----
            v.tensor_copy(
                ap_of(zf, ZF_OUT, [[N - 1, G], [1, N - 1]]),
                ap_of(banks, S_KR * D + N, [[D, G], [N, N - 1]]),
            )
            nc.sync.dma_start(
                ap_of(logs_d, 0, [[N - 1, G], [1, N - 1]], lead=[G * (N - 1), 128]),
                ap_of(zf, ZF_OUT, [[N - 1, G], [1, N - 1]]),
            )
            nc.sync.dma_start(
                ap_of(dsum_d, 0, [[1, G]], lead=[G, 128]),
                ap_of(zf, ZF_DSUM, [[1, G]]),
            )

    nc.compile()
    return nc


_NC_CACHE = {}


def get_nc():
    if "nc" not in _NC_CACHE:
        _NC_CACHE["nc"] = build_nc()
    return _NC_CACHE["nc"]


def _host_tables(trans_scores, dec_scores):
    """f32 coefficient tables (diag-packed [B, d, i]) + rho vectors + c0."""
    t = np.asarray(trans_scores, dtype=np.float32)
    dec = np.asarray(dec_scores, dtype=np.float32)
    B = t.shape[0]
    go = dec[..., 0]
    st = dec[..., 1]
    tm = np.where(t < -1e8, -np.inf, t).max(axis=3)
    with np.errstate(invalid="ignore"):
        colmax = tm.max(axis=1)
        proxy = np.nanmean(
            np.where(np.isfinite(colmax), colmax, np.nan)[:, 1:], axis=-1)
    c0 = np.clip(np.nan_to_num(proxy + 0.5), -20.0, 20.0).astype(np.float32)
    with np.errstate(under="ignore"):
        E = np.exp(t - c0[:, None, None, None])
        ego = np.exp(go)
        est = np.exp(st)
    d_idx, i_idx = np.meshgrid(np.arange(N), np.arange(N), indexing="ij")
    j_idx = np.minimum(i_idx + d_idx, N - 1)
    valid = ((i_idx + d_idx) <= N - 1)[None].astype(np.float32)
    ea = E[:, i_idx, j_idx, :]
    eb = E[:, j_idx, i_idx, :]
    a1 = ea[..., 1] * ego[:, :, 1, 1][:, i_idx] * valid
    a0 = ea[..., 0] * ego[:, :, 1, 0][:, i_idx] * valid
    b1 = eb[..., 1] * ego[:, :, 0, 1][:, j_idx] * valid
    b0 = eb[..., 0] * ego[:, :, 0, 0][:, j_idx] * valid
    srn, srh = est[:, :, 1, 0], est[:, :, 1, 1]
    sln, slh = est[:, :, 0, 0], est[:, :, 0, 1]
    slh_j = slh[:, j_idx] * valid
    sln_j = sln[:, j_idx] * valid
    srh_j = srh[:, j_idx] * valid
    srh_i, srn_i, slh_i = srh[:, i_idx], srn[:, i_idx], slh[:, i_idx]
    PA1 = a1 * slh_j * srh_j
    PA0 = a0 * slh_j * srh_j
    PA3 = a1 * sln_j * srh_j
    PB1 = b1 * srh_i * slh_i * valid
    PB0 = b1 * srn_i * slh_i * valid
    PB3 = b0 * srh_i * slh_i * valid
    PA0[:, 1] = a0[:, 1] * sln_j[:, 1] * srh_j[:, 1]
    PA1[:, 1] = 0.0
    PA3[:, 1] = 0.0
    PB0[:, 1] = b0[:, 1] * srn_i[:, 1] * slh_i[:, 1]
    PB1[:, 1] = 0.0
    PB3[:, 1] = 0.0
    rhoR = srn / srh
    rhoL = sln / slh
    return (PA1, PB1, PA0, PB0, PA3, PB3), (rhoR, rhoL), c0


def _short_ll(trans_scores, dec_scores, wmax=SHORT_LEN):
    """Exact f64 LL for len <= wmax: truncated exp-domain DP, direct
    reference recurrences (diag-packed [B, row=width, col=start])."""
    t = np.asarray(trans_scores, dtype=np.float64)
    dec = np.asarray(dec_scores, dtype=np.float64)
    B = t.shape[0]
    ego, est = np.exp(dec[..., 0]), np.exp(dec[..., 1])
    srn, srh = est[:, :, 1, 0], est[:, :, 1, 1]
    sln, slh = est[:, :, 0, 0], est[:, :, 0, 1]
    W = wmax + 1
    d_idx, i_idx = np.meshgrid(np.arange(W), np.arange(N), indexing="ij")
    j_idx = np.minimum(i_idx + d_idx, N - 1)
    valid = ((i_idx + d_idx) <= N - 1)[None].astype(np.float64)
    with np.errstate(under="ignore"):
        ea = np.exp(np.minimum(t[:, i_idx, j_idx, :], 700.0)) * valid[..., None]
        eb = np.exp(np.minimum(t[:, j_idx, i_idx, :], 700.0)) * valid[..., None]
    # arc*go factors, [B, W, N] indexed [w, i]
    ea1 = ea[..., 1] * ego[:, :, 1, 1][:, i_idx]
    ea0 = ea[..., 0] * ego[:, :, 1, 0][:, i_idx]
    eb1 = eb[..., 1] * ego[:, :, 0, 1][:, j_idx]
    eb0 = eb[..., 0] * ego[:, :, 0, 0][:, j_idx]
    KR = np.zeros((B, W, N)); KL = np.zeros((B, W, N))
    IR = np.zeros((B, W, N)); IL = np.zeros((B, W, N))
    KR[:, 0] = 1.0
    KL[:, 0] = 1.0
    for w in range(1, W):
        s = N - w
        ir = np.zeros((B, s)); il = np.zeros((B, s))
        for tq in range(w):
            aR = (ea1 if tq > 0 else ea0)[:, w, :s]
            bL = (eb1 if tq < w - 1 else eb0)[:, w, :s]
            stopCL = sln[:, w:w + s] if tq == w - 1 else slh[:, w:w + s]
            stopCR = srn[:, :s] if tq == 0 else srh[:, :s]
            krkl = KR[:, tq, :s] * KL[:, w - 1 - tq, 1 + tq:1 + tq + s]
            ir += krkl * aR * stopCL
            il += krkl * bL * stopCR
        IR[:, w, :s] = ir
        IL[:, w, :s] = il
        kr = np.zeros((B, s)); kl = np.zeros((B, s))
        for tq in range(w):
            stop2 = srn[:, w:w + s] if tq == w - 1 else srh[:, 1 + tq:1 + tq + s]
            kr += IR[:, tq + 1, :s] * KR[:, w - 1 - tq, 1 + tq:1 + tq + s] * stop2
            stop3 = sln[:, :s] if tq == 0 else slh[:, tq:tq + s]
            kl += KL[:, tq, :s] * stop3 * IL[:, w - tq, tq:tq + s]
        KR[:, w, :s] = kr
        KL[:, w, :s] = kl
    ll = np.full((B, W), np.nan)
    with np.errstate(divide="ignore"):
        for L in range(1, W):
            ll[:, L] = np.log(KR[:, L, 0] * srh[:, 0])
    return ll


def make_in_maps(trans_scores, dec_scores):
    (PA1, PB1, PA0, PB0, PA3, PB3), (rhoR, rhoL), c0 = _host_tables(
        trans_scores, dec_scores)
    B = PA1.shape[0]
    coefs = np.stack([PA1, PB1, PA0, PB0, PA3, PB3], axis=1)  # [B,6,41,41]
    coefs = coefs.reshape(B, COEF_IN).astype(BFNP)
    stops = np.stack([rhoR, rhoL], axis=1).reshape(B, STOP_IN).astype(BFNP)
    sll = _short_ll(trans_scores, dec_scores)
    in_maps = []
    for c in range(NCORES):
        sl = slice(c * B_CORE, (c + 1) * B_CORE)
        in_maps.append({"coefs": coefs[sl], "stops": stops[sl]})
    return in_maps, (c0, sll)


def assemble(results, len_array, extra):
    c0, sll = extra
    ln = np.asarray(len_array).astype(np.int64)
    c0 = np.asarray(c0).astype(np.float64)
    B = len(ln)
    out = np.empty(B, dtype=np.float32)
    for c, res in enumerate(results):
        ecr = res["ecr"].reshape(B_CORE, N - 1).astype(np.float64)
        dsum = res["dsum"].reshape(B_CORE).astype(np.float64)
        lc = ln[c * B_CORE:(c + 1) * B_CORE]
        idx = np.arange(B_CORE)
        with np.errstate(divide="ignore"):
            out[c * B_CORE:(c + 1) * B_CORE] = (
                np.log(ecr[idx, lc - 1]) + dsum * np.log(2.0) * lc
                + c0[c * B_CORE:(c + 1) * B_CORE] * lc
            ).astype(np.float32)
    short = ln <= SHORT_LEN
    gl = np.arange(B)
    out[short] = sll[gl[short], ln[short]].astype(np.float32)
    return out


def kernel(trans_scores, dec_scores, len_array):
    from concourse.bass_utils import run_bass_kernel_spmd

    nc = get_nc()
    in_maps, extra = make_in_maps(trans_scores, dec_scores)
    res = run_bass_kernel_spmd(nc, in_maps, core_ids=list(range(NCORES)))
    return assemble(res.results, len_array, extra)


# revision 7
# speedup vs baseline: 1.0800x; 1.0800x over previous
"""DMV inside algorithm (Eisner chart DP, logsumexp semiring) on Trainium2.

Strategy (v2)
-------------
Pure data parallelism: 4096 sentences -> 8 cores x 512; per core ONE pass of
[128 SBUF partitions] x [G=4 sentence groups], all tables bf16 so every big
DVE tensor_tensor runs in 2x_1p mode (0.52 ns/elem).

Exp-domain DP with a positive-only boundary decomposition (no cancellation,
bf16-safe). Tables per group, diag-packed (row = span width, col = start):
  KR/KL: complete-without-stop, row 0 == 1
  SIR[r,c] = eIR[r,c] * srh[c+r]  (right-incomplete, child's R-stop folded)
  SIL[r,c] = eIL[r,c] * slh[c]    (left-incomplete,  child's L-stop folded)
Recurrences per width w (s = 41-w):
  S[i]   = sum_{t=1..w-2} KR[t,i] * KL[w-1-t, i+1+t]          (shared!)
  SIR[w] = PA1*S + PA0*KL[w-1,i+1] + PA3*KR[w-1,i]
  SIL[w] = PB1*S + PB0*KL[w-1,i+1] + PB3*KR[w-1,i]
  KR[w]  = sum_{t=0..w-2} SIR[t+1,i]*KR[w-1-t,i+1+t] + SIR[w,i]*rhoR[i+w]
  KL[w]  = sum_{t=0..w-2} KL[t+1,i]*SIL[w-1-t,i+1+t] + SIL[w,i]*rhoL[i]
with all six coefficient tables and rhoR=srn/srh, rhoL=sln/slh positive,
precomputed on host. Reductions are in-place binary trees of bf16 adds.
Renorm at w=20 rescales row d by an exact power of two 2^(-k*d) (exponent
trick), k returned per sentence (dsum) and undone on host.

Host covers len<=6 sentences with an exact f64 mini-DP (the 2e-2 relative
gate implies tiny absolute budgets only for very short sentences).
"""

import os

os.environ.setdefault("JAX_PLATFORMS", "cpu")

import numpy as np
import ml_dtypes

import concourse.bass as bass  # noqa: F401  (registers engine classes)
import concourse.tile as tile
import bass_rust
from concourse import bacc, mybir

F32 = mybir.dt.float32
BF16 = mybir.dt.bfloat16
AF = mybir.ActivationFunctionType
OP = mybir.AluOpType
AX = mybir.AxisListType
BFNP = ml_dtypes.bfloat16

N = 41
D = 1681            # table pitch N*N
G = 4               # sentence groups per partition (1 pass = 512/core)
NCORES = 8
B_CORE = 128 * G
NTAB = 6            # coef tables: PA1, PB1, PA0, PB0, PA3, PB3
COEF_IN = NTAB * D
STOP_IN = 2 * N     # rhoR, rhoL
RENORM_W = 20
SHORT_LEN = 6       # host computes len <= SHORT_LEN exactly

# banks slots: 0..3 KR g0..3 | 4..7 SIL | 8..11 SIR | 12..15 KL
S_KR, S_SIL, S_SIR, S_KL = 0, 4, 8, 12

# bf16 scratch (element offsets)
PAP = 384           # per-group pitch of A-side product buffer (max (w-2)*s)
PBP = 424           # per-q pitch of B-side product buffer (max w*s)
ZB_PA = 0
ZB_PB = ZB_PA + 4 * PAP          # 1536
ZB_U0 = ZB_PB + 8 * PBP          # 4928
ZB_U3 = ZB_U0 + 2 * G * N
ZB_U03 = ZB_U3 + 2 * G * N
ZB_U1 = ZB_U03 + 2 * G * N
ZB_MB = ZB_U1 + 2 * G * N
ZB_MEXP = ZB_MB + G * 42
ZB_TOTAL = ZB_MEXP + G * 861

# f32 scratch
ZF_MU2 = 0          # [2,4]
ZF_MU = 8           # [4]
ZF_LM = 12          # [4]
ZF_SC = 16          # [4] 127-k
ZF_SCI = 20         # [4] bit-built 2^-k
ZF_DSUM = 24        # [4]
ZF_M = 28           # [4,42]
ZF_OUT = ZF_M + G * 42           # [4,40]
ZF_TOTAL = ZF_OUT + G * 40

LN2_32 = 32.0 * float(np.log(2.0))


def ap_of(t, offset, dims, lead=None):
    ap = t.copy()
    first = list(t.ap[0]) if lead is None else list(lead)
    ap.ap = bass_rust.VecI64Pair([first] + [list(d) for d in dims])
    ap.offset = offset
    return ap


def build_nc():
    nc = bacc.Bacc("TRN2", target_bir_lowering=False, debug=False, num_devices=1)
    coefs_in = nc.dram_tensor("coefs", [B_CORE, COEF_IN], BF16, kind="ExternalInput").ap()
    stops_in = nc.dram_tensor("stops", [B_CORE, STOP_IN], BF16, kind="ExternalInput").ap()
    logs_d = nc.dram_tensor("ecr", [B_CORE, N - 1], F32, kind="ExternalOutput").ap()
    dsum_d = nc.dram_tensor("dsum", [B_CORE], F32, kind="ExternalOutput").ap()

    with tile.TileContext(nc) as tc:
        with tc.tile_pool(name="p", bufs=1) as pool:
            banks_t = pool.tile([128, 16 * D], BF16)
            coefs_t = pool.tile([128, G * COEF_IN], BF16)
            stops_t = pool.tile([128, G * STOP_IN], BF16)
            zb_t = pool.tile([128, ZB_TOTAL], BF16)
            zf_t = pool.tile([128, ZF_TOTAL], F32)
            banks = banks_t[:]
            coefs = coefs_t[:]
            stops = stops_t[:]
            zb = zb_t[:]
            zf = zf_t[:]
            zi = zf.bitcast(mybir.dt.int32)

            v = nc.vector
            sc = nc.scalar

            # ---- input DMA ----
            # stops: dest [vec][g][41] <- dram row p*G+g cols vec*41+i
            for vec in range(2):
                nc.sync.dma_start(
                    ap_of(stops, vec * G * N, [[N, G], [1, N]]),
                    ap_of(stops_in, vec * N, [[STOP_IN, G], [1, N]],
                          lead=[G * STOP_IN, 128]),
                )
            # coefs: per group, row-chunked so later rows stream under the DP
            for lo, hi in ((0, 3 * N), (3 * N, 9 * N), (9 * N, 21 * N),
                           (21 * N, D)):
                for g in range(G):
                    nc.sync.dma_start(
                        ap_of(coefs, g * COEF_IN + lo, [[D, NTAB], [1, hi - lo]]),
                        ap_of(coefs_in, g * COEF_IN + lo, [[D, NTAB], [1, hi - lo]],
                              lead=[G * COEF_IN, 128]),
                    )

            # ---- init ----
            # zero rows 0..RENORM_W of all 16 slots (renorm reads full rows)
            nc.gpsimd.memset(
                ap_of(banks, 0, [[D, 16], [1, (RENORM_W + 1) * N]]), 0.0)
            v.memset(ap_of(zf, ZF_DSUM, [[1, G]]), 0.0)
            # KR[0,:] = KL[0,:] = 1
            v.memset(ap_of(banks, S_KR * D, [[12 * D, 2], [D, G], [1, N]]), 1.0)

            # ---- chart DP ----
            for w in range(1, N):
                s = N - w
                # A-side shared interior product + tree reduce
                if w >= 3:
                    na = w - 2
                    v.tensor_tensor(
                        ap_of(zb, ZB_PA, [[PAP, G], [s, na], [1, s]]),
                        ap_of(banks, S_KR * D + N, [[D, G], [N, na], [1, s]]),
                        ap_of(banks, S_KL * D + (w - 2) * N + 2,
                              [[D, G], [-(N - 1), na], [1, s]]),
                        OP.mult,
                    )
                    T = na
                    while T > 1:
                        h = T // 2
                        v.tensor_tensor(
                            ap_of(zb, ZB_PA, [[PAP, G], [s, h], [1, s]]),
                            ap_of(zb, ZB_PA, [[PAP, G], [s, h], [1, s]]),
                            ap_of(zb, ZB_PA + (T - h) * s,
                                  [[PAP, G], [s, h], [1, s]]),
                            OP.add,
                        )
                        T -= h
                # u0/u3 boundary terms; Pool engine once its fixed costs
                # hide under the DVE A-side product of the same width
                ub = nc.gpsimd if w >= 14 else v
                ub.tensor_tensor(
                    ap_of(zb, ZB_U0, [[G * N, 2], [N, G], [1, s]]),
                    ap_of(banks, S_KL * D + (w - 1) * N + 1,
                          [[0, 2], [D, G], [1, s]]),
                    ap_of(coefs, 2 * D + w * N, [[D, 2], [COEF_IN, G], [1, s]]),
                    OP.mult,
                )
                ub.tensor_tensor(
                    ap_of(zb, ZB_U3, [[G * N, 2], [N, G], [1, s]]),
                    ap_of(banks, S_KR * D + (w - 1) * N,
                          [[0, 2], [D, G], [1, s]]),
                    ap_of(coefs, 4 * D + w * N, [[D, 2], [COEF_IN, G], [1, s]]),
                    OP.mult,
                )
                wr_ap = ap_of(banks, S_SIR * D + w * N,
                              [[-4 * D, 2], [D, G], [1, s]])
                if w >= 3:
                    v.tensor_tensor(
                        ap_of(zb, ZB_U03, [[G * N, 2], [N, G], [1, s]]),
                        ap_of(zb, ZB_U0, [[G * N, 2], [N, G], [1, s]]),
                        ap_of(zb, ZB_U3, [[G * N, 2], [N, G], [1, s]]),
                        OP.add,
                    )
                    # u1 = {PA1,PB1}[w] * S
                    v.tensor_tensor(
                        ap_of(zb, ZB_U1, [[G * N, 2], [N, G], [1, s]]),
                        ap_of(zb, ZB_PA, [[0, 2], [PAP, G], [1, s]]),
                        ap_of(coefs, 0 * D + w * N, [[D, 2], [COEF_IN, G], [1, s]]),
                        OP.mult,
                    )
                    # SIR[w]/SIL[w] = u03 + u1
                    v.tensor_tensor(
                        wr_ap,
                        ap_of(zb, ZB_U03, [[G * N, 2], [N, G], [1, s]]),
                        ap_of(zb, ZB_U1, [[G * N, 2], [N, G], [1, s]]),
                        OP.add,
                    )
                else:
                    # SIR[w]/SIL[w] = u0 + u3 (no interior)
                    v.tensor_tensor(
                        wr_ap,
                        ap_of(zb, ZB_U0, [[G * N, 2], [N, G], [1, s]]),
                        ap_of(zb, ZB_U3, [[G * N, 2], [N, G], [1, s]]),
                        OP.add,
                    )
                # B side: correction slab (t = w-1) then interior products
                kout_ap = ap_of(banks, S_KR * D + w * N,
                                [[12 * D, 2], [D, G], [1, s]])
                corr_out = (
                    kout_ap if w == 1 else
                    ap_of(zb, ZB_PB + (w - 1) * s, [[4 * PBP, 2], [PBP, G], [1, s]])
                )
                cb = nc.gpsimd if w >= 6 else v
                cb.tensor_tensor(
                    corr_out,
                    ap_of(banks, S_SIR * D + w * N, [[-4 * D, 2], [D, G], [1, s]]),
                    ap_of(stops, w, [[G * N - w, 2], [N, G], [1, s]]),
                    OP.mult,
                )
                if w >= 2:
                    nb = w - 1
                    v.tensor_tensor(
                        ap_of(zb, ZB_PB, [[PBP, 8], [s, nb], [1, s]]),
                        ap_of(banks, S_SIR * D + N, [[D, 8], [N, nb], [1, s]]),
                        ap_of(banks, (w - 1) * N + 1, [[D, 8], [-(N - 1), nb], [1, s]]),
                        OP.mult,
                    )
                    T = w  # nb interior slabs + corr slab
                    while T > 2:
                        h = T // 2
                        v.tensor_tensor(
                            ap_of(zb, ZB_PB, [[PBP, 8], [s, h], [1, s]]),
                            ap_of(zb, ZB_PB, [[PBP, 8], [s, h], [1, s]]),
                            ap_of(zb, ZB_PB + (T - h) * s, [[PBP, 8], [s, h], [1, s]]),
                            OP.add,
                        )
                        T -= h
                    v.tensor_tensor(
                        kout_ap,
                        ap_of(zb, ZB_PB, [[4 * PBP, 2], [PBP, G], [1, s]]),
                        ap_of(zb, ZB_PB + s, [[4 * PBP, 2], [PBP, G], [1, s]]),
                        OP.add,
                    )

                if w == RENORM_W:
                    s0 = N - w
                    # mu[g] = max over KR/KL row w
                    v.reduce_max(
                        ap_of(zf, ZF_MU2, [[G, 2], [1, G]]),
                        ap_of(banks, S_KR * D + w * N, [[12 * D, 2], [D, G], [1, s0]]),
                        axis=AX.X,
                    )
                    v.tensor_tensor(
                        ap_of(zf, ZF_MU, [[1, G]]),
                        ap_of(zf, ZF_MU2, [[1, G]]),
                        ap_of(zf, ZF_MU2 + G, [[1, G]]),
                        OP.max,
                    )
                    v.tensor_scalar_mul(
                        ap_of(zf, ZF_MU, [[1, G]]), ap_of(zf, ZF_MU, [[1, G]]),
                        2.0 ** -32)
                    v.tensor_scalar_max(
                        ap_of(zf, ZF_MU, [[1, G]]), ap_of(zf, ZF_MU, [[1, G]]),
                        1e-36)
                    sc.activation(
                        ap_of(zf, ZF_LM, [[1, G]]), ap_of(zf, ZF_MU, [[1, G]]),
                        AF.Ln)
                    # k = round((ln(mu*2^-32) + 32 ln2) / (w ln2)); exact int
                    v.tensor_scalar(
                        ap_of(zf, ZF_LM, [[1, G]]), ap_of(zf, ZF_LM, [[1, G]]),
                        LN2_32, 1.0 / (w * float(np.log(2.0))),
                        OP.add, OP.mult,
                    )
                    v.tensor_scalar(
                        ap_of(zf, ZF_LM, [[1, G]]), ap_of(zf, ZF_LM, [[1, G]]),
                        12582912.0, 12582912.0, OP.add, OP.subtract,
                    )
                    v.tensor_tensor(
                        ap_of(zf, ZF_DSUM, [[1, G]]),
                        ap_of(zf, ZF_DSUM, [[1, G]]),
                        ap_of(zf, ZF_LM, [[1, G]]),
                        OP.add,
                    )
                    # 2^-k via exponent bits: (127 - k) << 23
                    v.tensor_scalar(
                        ap_of(zf, ZF_SC, [[1, G]]), ap_of(zf, ZF_LM, [[1, G]]),
                        -1.0, 127.0, OP.mult, OP.add,
                    )
                    v.tensor_copy(
                        ap_of(zi, ZF_SCI, [[1, G]]), ap_of(zf, ZF_SC, [[1, G]]))
                    v.tensor_scalar(
                        ap_of(zi, ZF_SCI, [[1, G]]), ap_of(zi, ZF_SCI, [[1, G]]),
                        23, None, OP.arith_shift_left,
                    )
                    # M[g,d] = 2^(-k d), multiplicative scan; then bf16 copy
                    v.memset(ap_of(zf, ZF_M, [[42, G], [1, 1]]), 1.0)
                    for g in range(G):
                        sca = ap_of(zf, ZF_SCI + g, [[0, N]])
                        v.tensor_tensor_scan(
                            ap_of(zf, ZF_M + g * 42 + 1, [[1, N]]),
                            sca, sca, 1.0, OP.mult, OP.bypass,
                        )
                    # M expanded to per-element rows (bf16, exact pow2),
                    # so the big rescale multiplies run in 2x_1p mode
                    v.tensor_copy(
                        ap_of(zb, ZB_MEXP, [[861, G], [1, (w + 1) * N]]),
                        ap_of(zf, ZF_M, [[42, G], [1, w + 1], [0, N]]))
                    for blk in range(4):
                        tap = ap_of(banks, blk * 4 * D,
                                    [[D, G], [1, (w + 1) * N]])
                        v.tensor_tensor(
                            tap, tap,
                            ap_of(zb, ZB_MEXP, [[861, G], [1, (w + 1) * N]]),
                            OP.mult,
                        )
                    # coef rows w+1..40: one extra arc factor 2^-k per group
                    for g in range(G):
                        cap = ap_of(coefs, g * COEF_IN + (w + 1) * N,
                                    [[D, NTAB], [1, (N - 1 - w) * N]])
                        v.tensor_scalar(
                            cap, cap,
                            ap_of(zf, ZF_M + g * 42 + 1, [[1, 1]]), None,
                            OP.mult,
                        )

            # ---- output: KR col 0 rows 1..40 (raw exp domain) + dsum ----
            v.tensor_copy(
                ap_of(zf, ZF_OUT, [[N - 1, G], [1, N - 1]]),
                ap_of(banks, S_KR * D + N, [[D, G], [N, N - 1]]),
            )
            nc.sync.dma_start(
                ap_of(logs_d, 0, [[N - 1, G], [1, N - 1]], lead=[G * (N - 1), 128]),
                ap_of(zf, ZF_OUT, [[N - 1, G], [1, N - 1]]),
            )
            nc.sync.dma_start(
                ap_of(dsum_d, 0, [[1, G]], lead=[G, 128]),
                ap_of(zf, ZF_DSUM, [[1, G]]),
            )

    nc.compile()
    return nc


_NC_CACHE = {}


def get_nc():
    if "nc" not in _NC_CACHE:
        _NC_CACHE["nc"] = build_nc()
    return _NC_CACHE["nc"]


def _host_tables(trans_scores, dec_scores):
    """f32 coefficient tables (diag-packed [B, d, i]) + rho vectors + c0."""
    t = np.asarray(trans_scores, dtype=np.float32)
    dec = np.asarray(dec_scores, dtype=np.float32)
    B = t.shape[0]
    go = dec[..., 0]
    st = dec[..., 1]
    tm = np.where(t < -1e8, -np.inf, t).max(axis=3)
    with np.errstate(invalid="ignore"):
        colmax = tm.max(axis=1)
        proxy = np.nanmean(
            np.where(np.isfinite(colmax), colmax, np.nan)[:, 1:], axis=-1)
    c0 = np.clip(np.nan_to_num(proxy + 0.5), -20.0, 20.0).astype(np.float32)
    with np.errstate(under="ignore"):
        E = np.exp(t - c0[:, None, None, None])
        ego = np.exp(go)
        est = np.exp(st)
    d_idx, i_idx = np.meshgrid(np.arange(N), np.arange(N), indexing="ij")
    j_idx = np.minimum(i_idx + d_idx, N - 1)
    valid = ((i_idx + d_idx) <= N - 1)[None].astype(np.float32)
    ea = E[:, i_idx, j_idx, :]
    eb = E[:, j_idx, i_idx, :]
    a1 = ea[..., 1] * ego[:, :, 1, 1][:, i_idx] * valid
    a0 = ea[..., 0] * ego[:, :, 1, 0][:, i_idx] * valid
    b1 = eb[..., 1] * ego[:, :, 0, 1][:, j_idx] * valid
    b0 = eb[..., 0] * ego[:, :, 0, 0][:, j_idx] * valid
    srn, srh = est[:, :, 1, 0], est[:, :, 1, 1]
    sln, slh = est[:, :, 0, 0], est[:, :, 0, 1]
    slh_j = slh[:, j_idx] * valid
    sln_j = sln[:, j_idx] * valid
    srh_j = srh[:, j_idx] * valid
    srh_i, srn_i, slh_i = srh[:, i_idx], srn[:, i_idx], slh[:, i_idx]
    PA1 = a1 * slh_j * srh_j
    PA0 = a0 * slh_j * srh_j
    PA3 = a1 * sln_j * srh_j
    PB1 = b1 * srh_i * slh_i * valid
    PB0 = b1 * srn_i * slh_i * valid
    PB3 = b0 * srh_i * slh_i * valid
    PA0[:, 1] = a0[:, 1] * sln_j[:, 1] * srh_j[:, 1]
    PA1[:, 1] = 0.0
    PA3[:, 1] = 0.0
    PB0[:, 1] = b0[:, 1] * srn_i[:, 1] * slh_i[:, 1]
    PB1[:, 1] = 0.0
    PB3[:, 1] = 0.0
    rhoR = srn / srh
    rhoL = sln / slh
    return (PA1, PB1, PA0, PB0, PA3, PB3), (rhoR, rhoL), c0


def _short_ll(trans_scores, dec_scores, wmax=SHORT_LEN):
    """Exact f64 LL for len <= wmax: truncated exp-domain DP, direct
    reference recurrences (diag-packed [B, row=width, col=start])."""
    t = np.asarray(trans_scores, dtype=np.float64)
    dec = np.asarray(dec_scores, dtype=np.float64)
    B = t.shape[0]
    ego, est = np.exp(dec[..., 0]), np.exp(dec[..., 1])
    srn, srh = est[:, :, 1, 0], est[:, :, 1, 1]
    sln, slh = est[:, :, 0, 0], est[:, :, 0, 1]
    W = wmax + 1
    d_idx, i_idx = np.meshgrid(np.arange(W), np.arange(N), indexing="ij")
    j_idx = np.minimum(i_idx + d_idx, N - 1)
    valid = ((i_idx + d_idx) <= N - 1)[None].astype(np.float64)
    with np.errstate(under="ignore"):
        ea = np.exp(np.minimum(t[:, i_idx, j_idx, :], 700.0)) * valid[..., None]
        eb = np.exp(np.minimum(t[:, j_idx, i_idx, :], 700.0)) * valid[..., None]
    # arc*go factors, [B, W, N] indexed [w, i]
    ea1 = ea[..., 1] * ego[:, :, 1, 1][:, i_idx]
    ea0 = ea[..., 0] * ego[:, :, 1, 0][:, i_idx]
    eb1 = eb[..., 1] * ego[:, :, 0, 1][:, j_idx]
    eb0 = eb[..., 0] * ego[:, :, 0, 0][:, j_idx]
    KR = np.zeros((B, W, N)); KL = np.zeros((B, W, N))
    IR = np.zeros((B, W, N)); IL = np.zeros((B, W, N))
    KR[:, 0] = 1.0
    KL[:, 0] = 1.0
    for w in range(1, W):
        s = N - w
        ir = np.zeros((B, s)); il = np.zeros((B, s))
        for tq in range(w):
            aR = (ea1 if tq > 0 else ea0)[:, w, :s]
            bL = (eb1 if tq < w - 1 else eb0)[:, w, :s]
            stopCL = sln[:, w:w + s] if tq == w - 1 else slh[:, w:w + s]
            stopCR = srn[:, :s] if tq == 0 else srh[:, :s]
            krkl = KR[:, tq, :s] * KL[:, w - 1 - tq, 1 + tq:1 + tq + s]
            ir += krkl * aR * stopCL
            il += krkl * bL * stopCR
        IR[:, w, :s] = ir
        IL[:, w, :s] = il
        kr = np.zeros((B, s)); kl = np.zeros((B, s))
        for tq in range(w):
            stop2 = srn[:, w:w + s] if tq == w - 1 else srh[:, 1 + tq:1 + tq + s]
            kr += IR[:, tq + 1, :s] * KR[:, w - 1 - tq, 1 + tq:1 + tq + s] * stop2
            stop3 = sln[:, :s] if tq == 0 else slh[:, tq:tq + s]
            kl += KL[:, tq, :s] * stop3 * IL[:, w - tq, tq:tq + s]
        KR[:, w, :s] = kr
        KL[:, w, :s] = kl
    ll = np.full((B, W), np.nan)
    with np.errstate(divide="ignore"):
        for L in range(1, W):
            ll[:, L] = np.log(KR[:, L, 0] * srh[:, 0])
    return ll


def make_in_maps(trans_scores, dec_scores):
    (PA1, PB1, PA0, PB0, PA3, PB3), (rhoR, rhoL), c0 = _host_tables(
        trans_scores, dec_scores)
    B = PA1.shape[0]
    coefs = np.stack([PA1, PB1, PA0, PB0, PA3, PB3], axis=1)  # [B,6,41,41]
    coefs = coefs.reshape(B, COEF_IN).astype(BFNP)
    stops = np.stack([rhoR, rhoL], axis=1).reshape(B, STOP_IN).astype(BFNP)
    sll = _short_ll(trans_scores, dec_scores)
    in_maps = []
    for c in range(NCORES):
        sl = slice(c * B_CORE, (c + 1) * B_CORE)
        in_maps.append({"coefs": coefs[sl], "stops": stops[sl]})
    return in_maps, (c0, sll)


def assemble(results, len_array, extra):
    c0, sll = extra
    ln = np.asarray(len_array).astype(np.int64)
    c0 = np.asarray(c0).astype(np.float64)
    B = len(ln)
    out = np.empty(B, dtype=np.float32)
    for c, res in enumerate(results):
        ecr = res["ecr"].reshape(B_CORE, N - 1).astype(np.float64)
        dsum = res["dsum"].reshape(B_CORE).astype(np.float64)
        lc = ln[c * B_CORE:(c + 1) * B_CORE]
        idx = np.arange(B_CORE)
        with np.errstate(divide="ignore"):
            out[c * B_CORE:(c + 1) * B_CORE] = (
                np.log(ecr[idx, lc - 1]) + dsum * np.log(2.0) * lc
                + c0[c * B_CORE:(c + 1) * B_CORE] * lc
            ).astype(np.float32)
    short = ln <= SHORT_LEN
    gl = np.arange(B)
    out[short] = sll[gl[short], ln[short]].astype(np.float32)
    return out


def kernel(trans_scores, dec_scores, len_array):
    from concourse.bass_utils import run_bass_kernel_spmd

    nc = get_nc()
    in_maps, extra = make_in_maps(trans_scores, dec_scores)
    res = run_bass_kernel_spmd(nc, in_maps, core_ids=list(range(NCORES)))
    return assemble(res.results, len_array, extra)


# revision 13
# speedup vs baseline: 1.2126x; 1.1228x over previous
"""DMV inside algorithm (Eisner chart DP, logsumexp semiring) on Trainium2.

Strategy (v2)
-------------
Pure data parallelism: 4096 sentences -> 8 cores x 512; per core ONE pass of
[128 SBUF partitions] x [G=4 sentence groups], all tables bf16 so every big
DVE tensor_tensor runs in 2x_1p mode (0.52 ns/elem).

Exp-domain DP with a positive-only boundary decomposition (no cancellation,
bf16-safe). Tables per group, diag-packed (row = span width, col = start):
  KR/KL: complete-without-stop, row 0 == 1
  SIR[r,c] = eIR[r,c] * srh[c+r]  (right-incomplete, child's R-stop folded)
  SIL[r,c] = eIL[r,c] * slh[c]    (left-incomplete,  child's L-stop folded)
Recurrences per width w (s = 41-w):
  S[i]   = sum_{t=1..w-2} KR[t,i] * KL[w-1-t, i+1+t]          (shared!)
  SIR[w] = PA1*S + PA0*KL[w-1,i+1] + PA3*KR[w-1,i]
  SIL[w] = PB1*S + PB0*KL[w-1,i+1] + PB3*KR[w-1,i]
  KR[w]  = sum_{t=0..w-2} SIR[t+1,i]*KR[w-1-t,i+1+t] + SIR[w,i]*rhoR[i+w]
  KL[w]  = sum_{t=0..w-2} KL[t+1,i]*SIL[w-1-t,i+1+t] + SIL[w,i]*rhoL[i]
with all six coefficient tables and rhoR=srn/srh, rhoL=sln/slh positive,
precomputed on host. Reductions are in-place binary trees of bf16 adds.
Renorm at w=20 rescales row d by an exact power of two 2^(-k*d) (exponent
trick), k returned per sentence (dsum) and undone on host.

Host covers len<=6 sentences with an exact f64 mini-DP (the 2e-2 relative
gate implies tiny absolute budgets only for very short sentences).
"""

import os

os.environ.setdefault("JAX_PLATFORMS", "cpu")

import numpy as np
import ml_dtypes

import concourse.bass as bass  # noqa: F401  (registers engine classes)
import concourse.tile as tile
import bass_rust
from concourse import bacc, mybir

F32 = mybir.dt.float32
BF16 = mybir.dt.bfloat16
AF = mybir.ActivationFunctionType
OP = mybir.AluOpType
AX = mybir.AxisListType
BFNP = ml_dtypes.bfloat16

N = 41
D = 1681            # table pitch N*N
G = 4               # sentence groups per partition (1 pass = 512/core)
NCORES = 8
B_CORE = 128 * G
NTAB = 6            # coef tables: PA1, PB1, PA0, PB0, PA3, PB3
COEF_IN = NTAB * D
CROW = NTAB * G * N  # coefs are [row][table][g][41]: one width's coefficient
                     # read is a single compact interval, so the Tile checker
                     # links it to exactly one streaming DMA chunk
STOP_IN = 2 * N     # rhoR, rhoL
RENORM_W = 20
SHORT_LEN = 6       # host computes len <= SHORT_LEN exactly

# banks slots: 0..3 KR g0..3 | 4..7 SIL | 8..11 SIR | 12..15 KL
S_KR, S_SIL, S_SIR, S_KL = 0, 4, 8, 12

# bf16 scratch (element offsets)
PAP = 384           # per-group pitch of A-side product buffer (max (w-2)*s)
PBP = 424           # per-q pitch of B-side product buffer (max w*s)
ZB_PA = 0
ZB_PB = ZB_PA + 4 * PAP          # 1536
ZB_U0 = ZB_PB + 8 * PBP          # 4928
ZB_U3 = ZB_U0 + 2 * G * N
ZB_U03 = ZB_U3 + 2 * G * N
ZB_U1 = ZB_U03 + 2 * G * N
ZB_CORR = ZB_U1 + 2 * G * N
ZB_MB = ZB_CORR + 8 * N
ZB_MEXP = ZB_MB + G * 42
ZB_TOTAL = ZB_MEXP + G * 861

# f32 scratch
ZF_MU2 = 0          # [2,4]
ZF_MU = 8           # [4]
ZF_LM = 12          # [4]
ZF_SC = 16          # [4] 127-k
ZF_SCI = 20         # [4] bit-built 2^-k
ZF_DSUM = 24        # [4]
ZF_M = 28           # [4,42]
ZF_OUT = ZF_M + G * 42           # [4,40]
ZF_TOTAL = ZF_OUT + G * 40

LN2_32 = 32.0 * float(np.log(2.0))


def ap_of(t, offset, dims, lead=None):
    ap = t.copy()
    first = list(t.ap[0]) if lead is None else list(lead)
    ap.ap = bass_rust.VecI64Pair([first] + [list(d) for d in dims])
    ap.offset = offset
    return ap


def build_nc():
    nc = bacc.Bacc("TRN2", target_bir_lowering=False, debug=False, num_devices=1)
    coefs_in = nc.dram_tensor("coefs", [128, G * COEF_IN], BF16, kind="ExternalInput").ap()
    stops_in = nc.dram_tensor("stops", [128, G * STOP_IN], BF16, kind="ExternalInput").ap()
    logs_d = nc.dram_tensor("ecr", [B_CORE, N - 1], F32, kind="ExternalOutput").ap()
    dsum_d = nc.dram_tensor("dsum", [B_CORE], F32, kind="ExternalOutput").ap()

    with tile.TileContext(nc) as tc:
        with tc.tile_pool(name="p", bufs=1) as pool:
            banks_t = pool.tile([128, 16 * D], BF16)
            coefs_t = pool.tile([128, G * COEF_IN], BF16)
            stops_t = pool.tile([128, G * STOP_IN], BF16)
            zb_t = pool.tile([128, ZB_TOTAL], BF16)
            zf_t = pool.tile([128, ZF_TOTAL], F32)
            banks = banks_t[:]
            coefs = coefs_t[:]
            stops = stops_t[:]
            zb = zb_t[:]
            zf = zf_t[:]
            zi = zf.bitcast(mybir.dt.int32)

            v = nc.vector
            sc = nc.scalar

            # ---- input DMA ----
            # coefs are row-interleaved ([g][row][6 tables][41]) so each
            # row-range chunk is one contiguous-per-group DMA with large
            # descriptors; chunks sized so arrival tracks DP consumption.
            chunks = ((1, 3), (3, 6), (6, 10), (10, 17), (17, 41))
            lo, hi = chunks[0]
            nc.sync.dma_start(
                ap_of(coefs, lo * CROW, [[1, (hi - lo) * CROW]]),
                ap_of(coefs_in, lo * CROW, [[1, (hi - lo) * CROW]],
                      lead=[G * COEF_IN, 128]),
            )
            nc.sync.dma_start(
                ap_of(stops, 0, [[1, G * STOP_IN]]),
                ap_of(stops_in, 0, [[1, G * STOP_IN]], lead=[G * STOP_IN, 128]),
            )
            for lo, hi in chunks[1:]:
                nc.sync.dma_start(
                    ap_of(coefs, lo * CROW, [[1, (hi - lo) * CROW]]),
                    ap_of(coefs_in, lo * CROW, [[1, (hi - lo) * CROW]],
                          lead=[G * COEF_IN, 128]),
                )

            # ---- init ----
            # The DP never writes: SIR/SIL row 0, and cols > 40-r of row r.
            # The renorm rescale reads full rows <= RENORM_W, so zero exactly
            # those cells (small Pool ops, disjoint from all DP writes, so
            # the DVE never waits on them).
            nc.gpsimd.memset(ap_of(banks, S_SIL * D, [[D, 8], [1, N]]), 0.0)
            for r in range(1, RENORM_W + 1):
                nc.gpsimd.memset(
                    ap_of(banks, r * N + (N - r), [[D, 16], [1, r]]), 0.0)
            v.memset(ap_of(zf, ZF_DSUM, [[1, G]]), 0.0)
            # KR[0,:] = KL[0,:] = 1
            v.memset(ap_of(banks, S_KR * D, [[12 * D, 2], [D, G], [1, N]]), 1.0)

            # ---- chart DP ----
            for w in range(1, N):
                s = N - w
                # A-side shared interior product + tree reduce
                if w >= 3:
                    na = w - 2
                    v.tensor_tensor(
                        ap_of(zb, ZB_PA, [[PAP, G], [s, na], [1, s]]),
                        ap_of(banks, S_KR * D + N, [[D, G], [N, na], [1, s]]),
                        ap_of(banks, S_KL * D + (w - 2) * N + 2,
                              [[D, G], [-(N - 1), na], [1, s]]),
                        OP.mult,
                    )
                    T = na
                    while T > 1:
                        h = T // 2
                        v.tensor_tensor(
                            ap_of(zb, ZB_PA, [[PAP, G], [s, h], [1, s]]),
                            ap_of(zb, ZB_PA, [[PAP, G], [s, h], [1, s]]),
                            ap_of(zb, ZB_PA + (T - h) * s,
                                  [[PAP, G], [s, h], [1, s]]),
                            OP.add,
                        )
                        T -= h
                # u0/u3 boundary terms; Pool engine once its fixed costs
                # hide under the DVE A-side product of the same width
                ub = nc.gpsimd if 14 <= w <= 32 else v
                ub.tensor_tensor(
                    ap_of(zb, ZB_U0, [[G * N, 2], [N, G], [1, s]]),
                    ap_of(banks, S_KL * D + (w - 1) * N + 1,
                          [[0, 2], [D, G], [1, s]]),
                    ap_of(coefs, w * CROW + 2 * G * N,
                          [[G * N, 2], [N, G], [1, s]]),
                    OP.mult,
                )
                ub.tensor_tensor(
                    ap_of(zb, ZB_U3, [[G * N, 2], [N, G], [1, s]]),
                    ap_of(banks, S_KR * D + (w - 1) * N,
                          [[0, 2], [D, G], [1, s]]),
                    ap_of(coefs, w * CROW + 4 * G * N,
                          [[G * N, 2], [N, G], [1, s]]),
                    OP.mult,
                )
                wr_ap = ap_of(banks, S_SIR * D + w * N,
                              [[-4 * D, 2], [D, G], [1, s]])
                if w >= 3:
                    v.tensor_tensor(
                        ap_of(zb, ZB_U03, [[G * N, 2], [N, G], [1, s]]),
                        ap_of(zb, ZB_U0, [[G * N, 2], [N, G], [1, s]]),
                        ap_of(zb, ZB_U3, [[G * N, 2], [N, G], [1, s]]),
                        OP.add,
                    )
                    # u1 = {PA1,PB1}[w] * S
                    v.tensor_tensor(
                        ap_of(zb, ZB_U1, [[G * N, 2], [N, G], [1, s]]),
                        ap_of(zb, ZB_PA, [[0, 2], [PAP, G], [1, s]]),
                        ap_of(coefs, w * CROW,
                              [[G * N, 2], [N, G], [1, s]]),
                        OP.mult,
                    )
                    # SIR[w]/SIL[w] = u03 + u1
                    v.tensor_tensor(
                        wr_ap,
                        ap_of(zb, ZB_U03, [[G * N, 2], [N, G], [1, s]]),
                        ap_of(zb, ZB_U1, [[G * N, 2], [N, G], [1, s]]),
                        OP.add,
                    )
                else:
                    # SIR[w]/SIL[w] = u0 + u3 (no interior)
                    v.tensor_tensor(
                        wr_ap,
                        ap_of(zb, ZB_U0, [[G * N, 2], [N, G], [1, s]]),
                        ap_of(zb, ZB_U3, [[G * N, 2], [N, G], [1, s]]),
                        OP.add,
                    )
                # B side: correction slab (t = w-1) then interior products
                kout_ap = ap_of(banks, S_KR * D + w * N,
                                [[12 * D, 2], [D, G], [1, s]])
                corr_out = (
                    kout_ap if w == 1 else
                    ap_of(zb, ZB_CORR, [[G * N, 2], [N, G], [1, s]])
                )
                cb = nc.gpsimd if w >= 6 else v
                cb.tensor_tensor(
                    corr_out,
                    ap_of(banks, S_SIR * D + w * N, [[-4 * D, 2], [D, G], [1, s]]),
                    ap_of(stops, w, [[G * N - w, 2], [N, G], [1, s]]),
                    OP.mult,
                )
                if w >= 2:
                    nb = w - 1
                    v.tensor_tensor(
                        ap_of(zb, ZB_PB, [[PBP, 8], [s, nb], [1, s]]),
                        ap_of(banks, S_SIR * D + N, [[D, 8], [N, nb], [1, s]]),
                        ap_of(banks, (w - 1) * N + 1, [[D, 8], [-(N - 1), nb], [1, s]]),
                        OP.mult,
                    )
                    T = nb
                    while T > 1:
                        h = T // 2
                        v.tensor_tensor(
                            ap_of(zb, ZB_PB, [[PBP, 8], [s, h], [1, s]]),
                            ap_of(zb, ZB_PB, [[PBP, 8], [s, h], [1, s]]),
                            ap_of(zb, ZB_PB + (T - h) * s, [[PBP, 8], [s, h], [1, s]]),
                            OP.add,
                        )
                        T -= h
                    v.tensor_tensor(
                        kout_ap,
                        ap_of(zb, ZB_PB, [[4 * PBP, 2], [PBP, G], [1, s]]),
                        ap_of(zb, ZB_CORR, [[G * N, 2], [N, G], [1, s]]),
                        OP.add,
                    )

                if w == RENORM_W:
                    s0 = N - w
                    # mu[g] = max over KR/KL row w
                    v.reduce_max(
                        ap_of(zf, ZF_MU2, [[G, 2], [1, G]]),
                        ap_of(banks, S_KR * D + w * N, [[12 * D, 2], [D, G], [1, s0]]),
                        axis=AX.X,
                    )
                    v.tensor_tensor(
                        ap_of(zf, ZF_MU, [[1, G]]),
                        ap_of(zf, ZF_MU2, [[1, G]]),
                        ap_of(zf, ZF_MU2 + G, [[1, G]]),
                        OP.max,
                    )
                    # k = round(log2(mu)/w) via exponent bits: the float
                    # bit pattern X of mu gives log2(mu) ~= X/2^23 - 127
                    # (max err 0.086, absorbed by the rounding)
                    v.tensor_copy(
                        ap_of(zf, ZF_LM, [[1, G]]),
                        ap_of(zi, ZF_MU, [[1, G]]))
                    v.tensor_scalar(
                        ap_of(zf, ZF_LM, [[1, G]]), ap_of(zf, ZF_LM, [[1, G]]),
                        1.0 / (w * 2.0 ** 23), -127.0 / w,
                        OP.mult, OP.add,
                    )
                    v.tensor_scalar(
                        ap_of(zf, ZF_LM, [[1, G]]), ap_of(zf, ZF_LM, [[1, G]]),
                        12582912.0, 12582912.0, OP.add, OP.subtract,
                    )
                    v.tensor_tensor(
                        ap_of(zf, ZF_DSUM, [[1, G]]),
                        ap_of(zf, ZF_DSUM, [[1, G]]),
                        ap_of(zf, ZF_LM, [[1, G]]),
                        OP.add,
                    )
                    # 2^-k via exponent bits: (127 - k) << 23
                    v.tensor_scalar(
                        ap_of(zf, ZF_SC, [[1, G]]), ap_of(zf, ZF_LM, [[1, G]]),
                        -1.0, 127.0, OP.mult, OP.add,
                    )
                    v.tensor_copy(
                        ap_of(zi, ZF_SCI, [[1, G]]), ap_of(zf, ZF_SC, [[1, G]]))
                    v.tensor_scalar(
                        ap_of(zi, ZF_SCI, [[1, G]]), ap_of(zi, ZF_SCI, [[1, G]]),
                        23, None, OP.arith_shift_left,
                    )
                    # M[g,d] = 2^(-k d), multiplicative scan; then bf16 copy
                    v.memset(ap_of(zf, ZF_M, [[42, G], [1, 1]]), 1.0)
                    for g in range(G):
                        sca = ap_of(zf, ZF_SCI + g, [[0, N]])
                        v.tensor_tensor_scan(
                            ap_of(zf, ZF_M + g * 42 + 1, [[1, N]]),
                            sca, sca, 1.0, OP.mult, OP.bypass,
                        )
                    # M expanded to per-element rows (bf16, exact pow2),
                    # so the big rescale multiplies run in 2x_1p mode
                    v.tensor_copy(
                        ap_of(zb, ZB_MEXP, [[861, G], [1, (w + 1) * N]]),
                        ap_of(zf, ZF_M, [[42, G], [1, w + 1], [0, N]]))
                    for blk in range(4):
                        tap = ap_of(banks, blk * 4 * D,
                                    [[D, G], [1, (w + 1) * N]])
                        v.tensor_tensor(
                            tap, tap,
                            ap_of(zb, ZB_MEXP, [[861, G], [1, (w + 1) * N]]),
                            OP.mult,
                        )
                    # coef rows w+1..40: one extra arc factor 2^-k per group
                    for g in range(G):
                        cap = ap_of(coefs, (w + 1) * CROW + g * N,
                                    [[CROW, N - 1 - w], [G * N, NTAB], [1, N]])
                        v.tensor_scalar(
                            cap, cap,
                            ap_of(zf, ZF_M + g * 42 + 1, [[1, 1]]), None,
                            OP.mult,
                        )
                    # rows <= RENORM_W of KR are final now: ship them early
                    v.tensor_copy(
                        ap_of(zf, ZF_OUT, [[N - 1, G], [1, RENORM_W]]),
                        ap_of(banks, S_KR * D + N, [[D, G], [N, RENORM_W]]),
                    )
                    nc.sync.dma_start(
                        ap_of(logs_d, 0, [[N - 1, G], [1, RENORM_W]],
                              lead=[G * (N - 1), 128]),
                        ap_of(zf, ZF_OUT, [[N - 1, G], [1, RENORM_W]]),
                    )
                    # dsum is final after the (single) renorm: ship it now
                    nc.sync.dma_start(
                        ap_of(dsum_d, 0, [[1, G]], lead=[G, 128]),
                        ap_of(zf, ZF_DSUM, [[1, G]]),
                    )

            # ---- output: KR col 0 rows 21..40 (rows 1..20 shipped at
            # renorm time) + dsum ----
            v.tensor_copy(
                ap_of(zf, ZF_OUT + RENORM_W, [[N - 1, G], [1, N - 1 - RENORM_W]]),
                ap_of(banks, S_KR * D + (RENORM_W + 1) * N,
                      [[D, G], [N, N - 1 - RENORM_W]]),
            )
            nc.sync.dma_start(
                ap_of(logs_d, RENORM_W, [[N - 1, G], [1, N - 1 - RENORM_W]],
                      lead=[G * (N - 1), 128]),
                ap_of(zf, ZF_OUT + RENORM_W, [[N - 1, G], [1, N - 1 - RENORM_W]]),
            )

    nc.compile()
    return nc


_NC_CACHE = {}


def get_nc():
    if "nc" not in _NC_CACHE:
        _NC_CACHE["nc"] = build_nc()
    return _NC_CACHE["nc"]


def _host_tables(trans_scores, dec_scores):
    """f32 coefficient tables (diag-packed [B, d, i]) + rho vectors + c0."""
    t = np.asarray(trans_scores, dtype=np.float32)
    dec = np.asarray(dec_scores, dtype=np.float32)
    B = t.shape[0]
    go = dec[..., 0]
    st = dec[..., 1]
    tm = np.where(t < -1e8, -np.inf, t).max(axis=3)
    with np.errstate(invalid="ignore"):
        colmax = tm.max(axis=1)
        proxy = np.nanmean(
            np.where(np.isfinite(colmax), colmax, np.nan)[:, 1:], axis=-1)
    c0 = np.clip(np.nan_to_num(proxy + 0.5), -20.0, 20.0).astype(np.float32)
    with np.errstate(under="ignore"):
        E = np.exp(t - c0[:, None, None, None])
        ego = np.exp(go)
        est = np.exp(st)
    d_idx, i_idx = np.meshgrid(np.arange(N), np.arange(N), indexing="ij")
    j_idx = np.minimum(i_idx + d_idx, N - 1)
    valid = ((i_idx + d_idx) <= N - 1)[None].astype(np.float32)
    ea = E[:, i_idx, j_idx, :]
    eb = E[:, j_idx, i_idx, :]
    a1 = ea[..., 1] * ego[:, :, 1, 1][:, i_idx] * valid
    a0 = ea[..., 0] * ego[:, :, 1, 0][:, i_idx] * valid
    b1 = eb[..., 1] * ego[:, :, 0, 1][:, j_idx] * valid
    b0 = eb[..., 0] * ego[:, :, 0, 0][:, j_idx] * valid
    srn, srh = est[:, :, 1, 0], est[:, :, 1, 1]
    sln, slh = est[:, :, 0, 0], est[:, :, 0, 1]
    slh_j = slh[:, j_idx] * valid
    sln_j = sln[:, j_idx] * valid
    srh_j = srh[:, j_idx] * valid
    srh_i, srn_i, slh_i = srh[:, i_idx], srn[:, i_idx], slh[:, i_idx]
    PA1 = a1 * slh_j * srh_j
    PA0 = a0 * slh_j * srh_j
    PA3 = a1 * sln_j * srh_j
    PB1 = b1 * srh_i * slh_i * valid
    PB0 = b1 * srn_i * slh_i * valid
    PB3 = b0 * srh_i * slh_i * valid
    PA0[:, 1] = a0[:, 1] * sln_j[:, 1] * srh_j[:, 1]
    PA1[:, 1] = 0.0
    PA3[:, 1] = 0.0
    PB0[:, 1] = b0[:, 1] * srn_i[:, 1] * slh_i[:, 1]
    PB1[:, 1] = 0.0
    PB3[:, 1] = 0.0
    rhoR = srn / srh
    rhoL = sln / slh
    return (PA1, PB1, PA0, PB0, PA3, PB3), (rhoR, rhoL), c0


def _short_ll(trans_scores, dec_scores, wmax=SHORT_LEN):
    """Exact f64 LL for len <= wmax: truncated exp-domain DP, direct
    reference recurrences (diag-packed [B, row=width, col=start])."""
    t = np.asarray(trans_scores, dtype=np.float64)
    dec = np.asarray(dec_scores, dtype=np.float64)
    B = t.shape[0]
    ego, est = np.exp(dec[..., 0]), np.exp(dec[..., 1])
    srn, srh = est[:, :, 1, 0], est[:, :, 1, 1]
    sln, slh = est[:, :, 0, 0], est[:, :, 0, 1]
    W = wmax + 1
    d_idx, i_idx = np.meshgrid(np.arange(W), np.arange(N), indexing="ij")
    j_idx = np.minimum(i_idx + d_idx, N - 1)
    valid = ((i_idx + d_idx) <= N - 1)[None].astype(np.float64)
    with np.errstate(under="ignore"):
        ea = np.exp(np.minimum(t[:, i_idx, j_idx, :], 700.0)) * valid[..., None]
        eb = np.exp(np.minimum(t[:, j_idx, i_idx, :], 700.0)) * valid[..., None]
    # arc*go factors, [B, W, N] indexed [w, i]
    ea1 = ea[..., 1] * ego[:, :, 1, 1][:, i_idx]
    ea0 = ea[..., 0] * ego[:, :, 1, 0][:, i_idx]
    eb1 = eb[..., 1] * ego[:, :, 0, 1][:, j_idx]
    eb0 = eb[..., 0] * ego[:, :, 0, 0][:, j_idx]
    KR = np.zeros((B, W, N)); KL = np.zeros((B, W, N))
    IR = np.zeros((B, W, N)); IL = np.zeros((B, W, N))
    KR[:, 0] = 1.0
    KL[:, 0] = 1.0
    for w in range(1, W):
        s = N - w
        ir = np.zeros((B, s)); il = np.zeros((B, s))
        for tq in range(w):
            aR = (ea1 if tq > 0 else ea0)[:, w, :s]
            bL = (eb1 if tq < w - 1 else eb0)[:, w, :s]
            stopCL = sln[:, w:w + s] if tq == w - 1 else slh[:, w:w + s]
            stopCR = srn[:, :s] if tq == 0 else srh[:, :s]
            krkl = KR[:, tq, :s] * KL[:, w - 1 - tq, 1 + tq:1 + tq + s]
            ir += krkl * aR * stopCL
            il += krkl * bL * stopCR
        IR[:, w, :s] = ir
        IL[:, w, :s] = il
        kr = np.zeros((B, s)); kl = np.zeros((B, s))
        for tq in range(w):
            stop2 = srn[:, w:w + s] if tq == w - 1 else srh[:, 1 + tq:1 + tq + s]
            kr += IR[:, tq + 1, :s] * KR[:, w - 1 - tq, 1 + tq:1 + tq + s] * stop2
            stop3 = sln[:, :s] if tq == 0 else slh[:, tq:tq + s]
            kl += KL[:, tq, :s] * stop3 * IL[:, w - tq, tq:tq + s]
        KR[:, w, :s] = kr
        KL[:, w, :s] = kl
    ll = np.full((B, W), np.nan)
    with np.errstate(divide="ignore"):
        for L in range(1, W):
            ll[:, L] = np.log(KR[:, L, 0] * srh[:, 0])
    return ll


def make_in_maps(trans_scores, dec_scores):
    (PA1, PB1, PA0, PB0, PA3, PB3), (rhoR, rhoL), c0 = _host_tables(
        trans_scores, dec_scores)
    B = PA1.shape[0]
    coefs = np.stack([PA1, PB1, PA0, PB0, PA3, PB3], axis=2)  # [B,41,6,41]
    coefs = coefs.reshape(B, COEF_IN).astype(BFNP)
    stops = np.stack([rhoR, rhoL], axis=1).astype(BFNP)  # [B,2,41]
    sll = _short_ll(trans_scores, dec_scores)
    in_maps = []
    for c in range(NCORES):
        sl = slice(c * B_CORE, (c + 1) * B_CORE)
        # dram row = partition, packed [row][table][g][41] like SBUF
        cc = np.ascontiguousarray(
            coefs[sl].reshape(128, G, N, NTAB, N).transpose(0, 2, 3, 1, 4)
        ).reshape(128, G * COEF_IN)
        ss = np.ascontiguousarray(
            stops[sl].reshape(128, G, 2, N).transpose(0, 2, 1, 3)
        ).reshape(128, G * STOP_IN)
        in_maps.append({"coefs": cc, "stops": ss})
    return in_maps, (c0, sll)


def assemble(results, len_array, extra):
    c0, sll = extra
    ln = np.asarray(len_array).astype(np.int64)
    c0 = np.asarray(c0).astype(np.float64)
    B = len(ln)
    out = np.empty(B, dtype=np.float32)
    for c, res in enumerate(results):
        ecr = res["ecr"].reshape(B_CORE, N - 1).astype(np.float64)
        dsum = res["dsum"].reshape(B_CORE).astype(np.float64)
        lc = ln[c * B_CORE:(c + 1) * B_CORE]
        idx = np.arange(B_CORE)
        with np.errstate(divide="ignore"):
            out[c * B_CORE:(c + 1) * B_CORE] = (
                np.log(ecr[idx, lc - 1]) + dsum * np.log(2.0) * lc
                + c0[c * B_CORE:(c + 1) * B_CORE] * lc
            ).astype(np.float32)
    short = ln <= SHORT_LEN
    gl = np.arange(B)
    out[short] = sll[gl[short], ln[short]].astype(np.float32)
    return out


def kernel(trans_scores, dec_scores, len_array):
    from concourse.bass_utils import run_bass_kernel_spmd

    nc = get_nc()
    in_maps, extra = make_in_maps(trans_scores, dec_scores)
    res = run_bass_kernel_spmd(nc, in_maps, core_ids=list(range(NCORES)))
    return assemble(res.results, len_array, extra)


# revision 21
# speedup vs baseline: 1.2662x; 1.0442x over previous
"""DMV inside algorithm (Eisner chart DP, logsumexp semiring) on Trainium2.

Strategy (v2)
-------------
Pure data parallelism: 4096 sentences -> 8 cores x 512; per core ONE pass of
[128 SBUF partitions] x [G=4 sentence groups], all tables bf16 so every big
DVE tensor_tensor runs in 2x_1p mode (0.52 ns/elem).

Exp-domain DP with a positive-only boundary decomposition (no cancellation,
bf16-safe). Tables per group, diag-packed (row = span width, col = start):
  KR/KL: complete-without-stop, row 0 == 1
  SIR[r,c] = eIR[r,c] * srh[c+r]  (right-incomplete, child's R-stop folded)
  SIL[r,c] = eIL[r,c] * slh[c]    (left-incomplete,  child's L-stop folded)
Recurrences per width w (s = 41-w):
  S[i]   = sum_{t=1..w-2} KR[t,i] * KL[w-1-t, i+1+t]          (shared!)
  SIR[w] = PA1*S + PA0*KL[w-1,i+1] + PA3*KR[w-1,i]
  SIL[w] = PB1*S + PB0*KL[w-1,i+1] + PB3*KR[w-1,i]
  KR[w]  = sum_{t=0..w-2} SIR[t+1,i]*KR[w-1-t,i+1+t] + SIR[w,i]*rhoR[i+w]
  KL[w]  = sum_{t=0..w-2} KL[t+1,i]*SIL[w-1-t,i+1+t] + SIL[w,i]*rhoL[i]
with all six coefficient tables and rhoR=srn/srh, rhoL=sln/slh positive,
precomputed on host. Reductions are in-place binary trees of bf16 adds.
Renorm at w=20 rescales row d by an exact power of two 2^(-k*d) (exponent
trick), k returned per sentence (dsum) and undone on host.

Host covers len<=6 sentences with an exact f64 mini-DP (the 2e-2 relative
gate implies tiny absolute budgets only for very short sentences).
"""

import os

os.environ.setdefault("JAX_PLATFORMS", "cpu")

import numpy as np
import ml_dtypes

import concourse.bass as bass  # noqa: F401  (registers engine classes)
import concourse.tile as tile
import bass_rust
from concourse import bacc, mybir

F32 = mybir.dt.float32
BF16 = mybir.dt.bfloat16
AF = mybir.ActivationFunctionType
OP = mybir.AluOpType
AX = mybir.AxisListType
BFNP = ml_dtypes.bfloat16

N = 41
D = 1681            # table pitch N*N
G = 4               # sentence groups per partition (1 pass = 512/core)
NCORES = 8
B_CORE = 128 * G
NTAB = 6            # coef tables: PA1, PB1, PA0, PB0, PA3, PB3
COEF_IN = NTAB * D
CROW = NTAB * G * N  # coefs are [row][table][g][41]: one width's coefficient
                     # read is a single compact interval, so the Tile checker
                     # links it to exactly one streaming DMA chunk
STOP_IN = 2 * N     # rhoR, rhoL
RENORM_W = 20
SHORT_LEN = 6       # host computes len <= SHORT_LEN exactly

# banks slots: 0..3 KR g0..3 | 4..7 SIL | 8..11 SIR | 12..15 KL
S_KR, S_SIL, S_SIR, S_KL = 0, 4, 8, 12

# bf16 scratch (element offsets)
PAP = 384           # per-group pitch of A-side product buffer (max (w-2)*s)
PBP = 424           # per-q pitch of B-side product buffer (max w*s)
ZB_PA = 0
ZB_PB = ZB_PA + 4 * PAP          # 1536
ZB_U0 = ZB_PB + 8 * PBP          # 4928
ZB_U3 = ZB_U0 + 2 * G * N
ZB_U03 = ZB_U3 + 2 * G * N
ZB_U1 = ZB_U03 + 2 * G * N
ZB_CORR = ZB_U1 + 2 * G * N
ZB_MB = ZB_CORR + 8 * N
ZB_MEXP = ZB_MB + G * 42
ZB_TOTAL = ZB_MEXP + G * 861

# f32 scratch
ZF_MU2 = 0          # [2,4]
ZF_MU = 8           # [4]
ZF_LM = 12          # [4]
ZF_SC = 16          # [4] 127-k
ZF_SCI = 20         # [4] bit-built 2^-k
ZF_DSUM = 24        # [4]
ZF_M = 28           # [4,42]
ZF_OUT = ZF_M + G * 42           # [4,40]
ZF_TOTAL = ZF_OUT + G * 40

LN2_32 = 32.0 * float(np.log(2.0))


def ap_of(t, offset, dims, lead=None):
    ap = t.copy()
    first = list(t.ap[0]) if lead is None else list(lead)
    ap.ap = bass_rust.VecI64Pair([first] + [list(d) for d in dims])
    ap.offset = offset
    return ap


def build_nc():
    nc = bacc.Bacc("TRN2", target_bir_lowering=False, debug=False, num_devices=1)
    coefs_in = nc.dram_tensor("coefs", [128, G * COEF_IN], BF16, kind="ExternalInput").ap()
    stops_in = nc.dram_tensor("stops", [128, G * STOP_IN], BF16, kind="ExternalInput").ap()
    logs_d = nc.dram_tensor("ecr", [B_CORE, N - 1], F32, kind="ExternalOutput").ap()
    dsum_d = nc.dram_tensor("dsum", [B_CORE], F32, kind="ExternalOutput").ap()

    with tile.TileContext(nc) as tc:
        with tc.tile_pool(name="p", bufs=1) as pool:
            banks_t = pool.tile([128, 16 * D], BF16)
            coefs_t = pool.tile([128, G * COEF_IN], BF16)
            stops_t = pool.tile([128, G * STOP_IN], BF16)
            zb_t = pool.tile([128, ZB_TOTAL], BF16)
            zf_t = pool.tile([128, ZF_TOTAL], F32)
            banks = banks_t[:]
            coefs = coefs_t[:]
            stops = stops_t[:]
            zb = zb_t[:]
            zf = zf_t[:]
            zi = zf.bitcast(mybir.dt.int32)

            v = nc.vector
            sc = nc.scalar

            # ---- input DMA ----
            # coefs are row-interleaved ([g][row][6 tables][41]) so each
            # row-range chunk is one contiguous-per-group DMA with large
            # descriptors; chunks sized so arrival tracks DP consumption.
            chunks = ((1, 2), (2, 4), (4, 8), (8, 16), (16, 41))
            lo, hi = chunks[0]
            nc.sync.dma_start(
                ap_of(coefs, lo * CROW, [[1, (hi - lo) * CROW]]),
                ap_of(coefs_in, lo * CROW, [[1, (hi - lo) * CROW]],
                      lead=[G * COEF_IN, 128]),
            )
            nc.sync.dma_start(
                ap_of(stops, 0, [[1, G * STOP_IN]]),
                ap_of(stops_in, 0, [[1, G * STOP_IN]], lead=[G * STOP_IN, 128]),
            )
            for lo, hi in chunks[1:]:
                nc.sync.dma_start(
                    ap_of(coefs, lo * CROW, [[1, (hi - lo) * CROW]]),
                    ap_of(coefs_in, lo * CROW, [[1, (hi - lo) * CROW]],
                          lead=[G * COEF_IN, 128]),
                )

            # ---- init ----
            # The DP never writes: SIR/SIL row 0, and cols > 40-r of row r.
            # The renorm rescale reads full rows <= RENORM_W, so zero exactly
            # those cells (small Pool ops, disjoint from all DP writes, so
            # the DVE never waits on them).
            nc.gpsimd.memset(ap_of(banks, S_SIL * D, [[D, 8], [1, N]]), 0.0)
            for r in range(1, RENORM_W + 1):
                nc.gpsimd.memset(
                    ap_of(banks, r * N + (N - r), [[D, 16], [1, r]]), 0.0)
            v.memset(ap_of(zf, ZF_DSUM, [[1, G]]), 0.0)
            # KR[0,:] = KL[0,:] = 1
            v.memset(ap_of(banks, S_KR * D, [[12 * D, 2], [D, G], [1, N]]), 1.0)

            # ---- chart DP ----
            for w in range(1, N):
                s = N - w
                # B-side interior products first: they only need width w-1
                # tables, and give the Pool boundary ops a large window
                if w >= 2:
                    nb = w - 1
                    v.tensor_tensor(
                        ap_of(zb, ZB_PB, [[PBP, 8], [s, nb], [1, s]]),
                        ap_of(banks, S_SIR * D + N, [[D, 8], [N, nb], [1, s]]),
                        ap_of(banks, (w - 1) * N + 1, [[D, 8], [-(N - 1), nb], [1, s]]),
                        OP.mult,
                    )
                # A-side shared interior product + tree reduce
                if w >= 3:
                    na = w - 2
                    v.tensor_tensor(
                        ap_of(zb, ZB_PA, [[PAP, G], [s, na], [1, s]]),
                        ap_of(banks, S_KR * D + N, [[D, G], [N, na], [1, s]]),
                        ap_of(banks, S_KL * D + (w - 2) * N + 2,
                              [[D, G], [-(N - 1), na], [1, s]]),
                        OP.mult,
                    )
                    T = na
                    while T > 1:
                        h = T // 2
                        v.tensor_tensor(
                            ap_of(zb, ZB_PA, [[PAP, G], [s, h], [1, s]]),
                            ap_of(zb, ZB_PA, [[PAP, G], [s, h], [1, s]]),
                            ap_of(zb, ZB_PA + (T - h) * s,
                                  [[PAP, G], [s, h], [1, s]]),
                            OP.add,
                        )
                        T -= h
                # u0/u3 boundary terms; Pool engine once its fixed costs
                # hide under the DVE A-side product of the same width
                ub = nc.gpsimd if w >= 6 else v
                ub.tensor_tensor(
                    ap_of(zb, ZB_U0, [[G * N, 2], [N, G], [1, s]]),
                    ap_of(banks, S_KL * D + (w - 1) * N + 1,
                          [[0, 2], [D, G], [1, s]]),
                    ap_of(coefs, w * CROW + 2 * G * N,
                          [[G * N, 2], [N, G], [1, s]]),
                    OP.mult,
                )
                ub.tensor_tensor(
                    ap_of(zb, ZB_U3, [[G * N, 2], [N, G], [1, s]]),
                    ap_of(banks, S_KR * D + (w - 1) * N,
                          [[0, 2], [D, G], [1, s]]),
                    ap_of(coefs, w * CROW + 4 * G * N,
                          [[G * N, 2], [N, G], [1, s]]),
                    OP.mult,
                )
                wr_ap = ap_of(banks, S_SIR * D + w * N,
                              [[-4 * D, 2], [D, G], [1, s]])
                if w >= 3:
                    v.tensor_tensor(
                        ap_of(zb, ZB_U03, [[G * N, 2], [N, G], [1, s]]),
                        ap_of(zb, ZB_U0, [[G * N, 2], [N, G], [1, s]]),
                        ap_of(zb, ZB_U3, [[G * N, 2], [N, G], [1, s]]),
                        OP.add,
                    )
                    # u1 = {PA1,PB1}[w] * S
                    v.tensor_tensor(
                        ap_of(zb, ZB_U1, [[G * N, 2], [N, G], [1, s]]),
                        ap_of(zb, ZB_PA, [[0, 2], [PAP, G], [1, s]]),
                        ap_of(coefs, w * CROW,
                              [[G * N, 2], [N, G], [1, s]]),
                        OP.mult,
                    )
                    # SIR[w]/SIL[w] = u03 + u1
                    v.tensor_tensor(
                        wr_ap,
                        ap_of(zb, ZB_U03, [[G * N, 2], [N, G], [1, s]]),
                        ap_of(zb, ZB_U1, [[G * N, 2], [N, G], [1, s]]),
                        OP.add,
                    )
                else:
                    # SIR[w]/SIL[w] = u0 + u3 (no interior)
                    v.tensor_tensor(
                        wr_ap,
                        ap_of(zb, ZB_U0, [[G * N, 2], [N, G], [1, s]]),
                        ap_of(zb, ZB_U3, [[G * N, 2], [N, G], [1, s]]),
                        OP.add,
                    )
                # B side: correction slab (t = w-1) then interior products
                kout_ap = ap_of(banks, S_KR * D + w * N,
                                [[12 * D, 2], [D, G], [1, s]])
                corr_out = (
                    kout_ap if w == 1 else
                    ap_of(zb, ZB_CORR, [[G * N, 2], [N, G], [1, s]])
                )
                cb = nc.gpsimd if w >= 6 else v
                cb.tensor_tensor(
                    corr_out,
                    ap_of(banks, S_SIR * D + w * N, [[-4 * D, 2], [D, G], [1, s]]),
                    ap_of(stops, w, [[G * N - w, 2], [N, G], [1, s]]),
                    OP.mult,
                )
                if w >= 2:
                    T = w - 1
                    while T > 1:
                        h = T // 2
                        v.tensor_tensor(
                            ap_of(zb, ZB_PB, [[PBP, 8], [s, h], [1, s]]),
                            ap_of(zb, ZB_PB, [[PBP, 8], [s, h], [1, s]]),
                            ap_of(zb, ZB_PB + (T - h) * s, [[PBP, 8], [s, h], [1, s]]),
                            OP.add,
                        )
                        T -= h
                    v.tensor_tensor(
                        kout_ap,
                        ap_of(zb, ZB_PB, [[4 * PBP, 2], [PBP, G], [1, s]]),
                        ap_of(zb, ZB_CORR, [[G * N, 2], [N, G], [1, s]]),
                        OP.add,
                    )

                if w == RENORM_W:
                    s0 = N - w
                    # mu[g] = max over KR/KL row w
                    v.reduce_max(
                        ap_of(zf, ZF_MU2, [[G, 2], [1, G]]),
                        ap_of(banks, S_KR * D + w * N, [[12 * D, 2], [D, G], [1, s0]]),
                        axis=AX.X,
                    )
                    v.tensor_tensor(
                        ap_of(zf, ZF_MU, [[1, G]]),
                        ap_of(zf, ZF_MU2, [[1, G]]),
                        ap_of(zf, ZF_MU2 + G, [[1, G]]),
                        OP.max,
                    )
                    # k = round(log2(mu)/w) via exponent bits: the float
                    # bit pattern X of mu gives log2(mu) ~= X/2^23 - 127
                    # (max err 0.086, absorbed by the rounding)
                    v.tensor_copy(
                        ap_of(zf, ZF_LM, [[1, G]]),
                        ap_of(zi, ZF_MU, [[1, G]]))
                    v.tensor_scalar(
                        ap_of(zf, ZF_LM, [[1, G]]), ap_of(zf, ZF_LM, [[1, G]]),
                        1.0 / (w * 2.0 ** 23), -127.0 / w,
                        OP.mult, OP.add,
                    )
                    v.tensor_scalar(
                        ap_of(zf, ZF_LM, [[1, G]]), ap_of(zf, ZF_LM, [[1, G]]),
                        12582912.0, 12582912.0, OP.add, OP.subtract,
                    )
                    v.tensor_tensor(
                        ap_of(zf, ZF_DSUM, [[1, G]]),
                        ap_of(zf, ZF_DSUM, [[1, G]]),
                        ap_of(zf, ZF_LM, [[1, G]]),
                        OP.add,
                    )
                    # 2^-k via exponent bits: (127 - k) << 23
                    v.tensor_scalar(
                        ap_of(zf, ZF_SC, [[1, G]]), ap_of(zf, ZF_LM, [[1, G]]),
                        -1.0, 127.0, OP.mult, OP.add,
                    )
                    v.tensor_copy(
                        ap_of(zi, ZF_SCI, [[1, G]]), ap_of(zf, ZF_SC, [[1, G]]))
                    v.tensor_scalar(
                        ap_of(zi, ZF_SCI, [[1, G]]), ap_of(zi, ZF_SCI, [[1, G]]),
                        23, None, OP.arith_shift_left,
                    )
                    # M[g,d] = 2^(-k d), multiplicative scan; then bf16 copy
                    v.memset(ap_of(zf, ZF_M, [[42, G], [1, 1]]), 1.0)
                    for g in range(G):
                        sca = ap_of(zf, ZF_SCI + g, [[0, N]])
                        v.tensor_tensor_scan(
                            ap_of(zf, ZF_M + g * 42 + 1, [[1, N]]),
                            sca, sca, 1.0, OP.mult, OP.bypass,
                        )
                    # M expanded to per-element rows (bf16, exact pow2),
                    # so the big rescale multiplies run in 2x_1p mode;
                    # expansion split across DVE/Pool to halve its latency
                    HB = 11 * N
                    v.tensor_copy(
                        ap_of(zb, ZB_MEXP, [[861, G], [1, HB]]),
                        ap_of(zf, ZF_M, [[42, G], [1, 11], [0, N]]))
                    nc.gpsimd.tensor_copy(
                        ap_of(zb, ZB_MEXP + HB, [[861, G], [1, (w + 1) * N - HB]]),
                        ap_of(zf, ZF_M + 11, [[42, G], [1, w - 10], [0, N]]))
                    for blk in range(4):
                        tap = ap_of(banks, blk * 4 * D,
                                    [[D, G], [1, (w + 1) * N]])
                        v.tensor_tensor(
                            tap, tap,
                            ap_of(zb, ZB_MEXP, [[861, G], [1, (w + 1) * N]]),
                            OP.mult,
                        )
                    # coef rows w+1..40: one extra arc factor 2^-k per group
                    for g in range(G):
                        cap = ap_of(coefs, (w + 1) * CROW + g * N,
                                    [[CROW, N - 1 - w], [G * N, NTAB], [1, N]])
                        v.tensor_scalar(
                            cap, cap,
                            ap_of(zf, ZF_M + g * 42 + 1, [[1, 1]]), None,
                            OP.mult,
                        )
                    # rows <= RENORM_W of KR are final now: ship them early
                    v.tensor_copy(
                        ap_of(zf, ZF_OUT, [[N - 1, G], [1, RENORM_W]]),
                        ap_of(banks, S_KR * D + N, [[D, G], [N, RENORM_W]]),
                    )
                    nc.sync.dma_start(
                        ap_of(logs_d, 0, [[N - 1, G], [1, RENORM_W]],
                              lead=[G * (N - 1), 128]),
                        ap_of(zf, ZF_OUT, [[N - 1, G], [1, RENORM_W]]),
                    )
                    # dsum is final after the (single) renorm: ship it now
                    nc.sync.dma_start(
                        ap_of(dsum_d, 0, [[1, G]], lead=[G, 128]),
                        ap_of(zf, ZF_DSUM, [[1, G]]),
                    )

            # ---- output: KR col 0 rows 21..40 (rows 1..20 shipped at
            # renorm time) ----
            v.tensor_copy(
                ap_of(zf, ZF_OUT + RENORM_W, [[N - 1, G], [1, N - 1 - RENORM_W]]),
                ap_of(banks, S_KR * D + (RENORM_W + 1) * N,
                      [[D, G], [N, N - 1 - RENORM_W]]),
            )
            nc.sync.dma_start(
                ap_of(logs_d, RENORM_W, [[N - 1, G], [1, N - 1 - RENORM_W]],
                      lead=[G * (N - 1), 128]),
                ap_of(zf, ZF_OUT + RENORM_W, [[N - 1, G], [1, N - 1 - RENORM_W]]),
            )

    nc.compile()
    return nc


_NC_CACHE = {}


def get_nc():
    if "nc" not in _NC_CACHE:
        _NC_CACHE["nc"] = build_nc()
    return _NC_CACHE["nc"]


def _host_tables(trans_scores, dec_scores):
    """f32 coefficient tables (diag-packed [B, d, i]) + rho vectors + c0."""
    t = np.asarray(trans_scores, dtype=np.float32)
    dec = np.asarray(dec_scores, dtype=np.float32)
    B = t.shape[0]
    go = dec[..., 0]
    st = dec[..., 1]
    tm = np.where(t < -1e8, -np.inf, t).max(axis=3)
    with np.errstate(invalid="ignore"):
        colmax = tm.max(axis=1)
        proxy = np.nanmean(
            np.where(np.isfinite(colmax), colmax, np.nan)[:, 1:], axis=-1)
    c0 = np.clip(np.nan_to_num(proxy + 0.5), -20.0, 20.0).astype(np.float32)
    with np.errstate(under="ignore"):
        E = np.exp(t - c0[:, None, None, None])
        ego = np.exp(go)
        est = np.exp(st)
    d_idx, i_idx = np.meshgrid(np.arange(N), np.arange(N), indexing="ij")
    j_idx = np.minimum(i_idx + d_idx, N - 1)
    valid = ((i_idx + d_idx) <= N - 1)[None].astype(np.float32)
    ea = E[:, i_idx, j_idx, :]
    eb = E[:, j_idx, i_idx, :]
    a1 = ea[..., 1] * ego[:, :, 1, 1][:, i_idx] * valid
    a0 = ea[..., 0] * ego[:, :, 1, 0][:, i_idx] * valid
    b1 = eb[..., 1] * ego[:, :, 0, 1][:, j_idx] * valid
    b0 = eb[..., 0] * ego[:, :, 0, 0][:, j_idx] * valid
    srn, srh = est[:, :, 1, 0], est[:, :, 1, 1]
    sln, slh = est[:, :, 0, 0], est[:, :, 0, 1]
    slh_j = slh[:, j_idx] * valid
    sln_j = sln[:, j_idx] * valid
    srh_j = srh[:, j_idx] * valid
    srh_i, srn_i, slh_i = srh[:, i_idx], srn[:, i_idx], slh[:, i_idx]
    PA1 = a1 * slh_j * srh_j
    PA0 = a0 * slh_j * srh_j
    PA3 = a1 * sln_j * srh_j
    PB1 = b1 * srh_i * slh_i * valid
    PB0 = b1 * srn_i * slh_i * valid
    PB3 = b0 * srh_i * slh_i * valid
    PA0[:, 1] = a0[:, 1] * sln_j[:, 1] * srh_j[:, 1]
    PA1[:, 1] = 0.0
    PA3[:, 1] = 0.0
    PB0[:, 1] = b0[:, 1] * srn_i[:, 1] * slh_i[:, 1]
    PB1[:, 1] = 0.0
    PB3[:, 1] = 0.0
    rhoR = srn / srh
    rhoL = sln / slh
    return (PA1, PB1, PA0, PB0, PA3, PB3), (rhoR, rhoL), c0


def _short_ll(trans_scores, dec_scores, wmax=SHORT_LEN):
    """Exact f64 LL for len <= wmax: truncated exp-domain DP, direct
    reference recurrences (diag-packed [B, row=width, col=start])."""
    t = np.asarray(trans_scores, dtype=np.float64)
    dec = np.asarray(dec_scores, dtype=np.float64)
    B = t.shape[0]
    ego, est = np.exp(dec[..., 0]), np.exp(dec[..., 1])
    srn, srh = est[:, :, 1, 0], est[:, :, 1, 1]
    sln, slh = est[:, :, 0, 0], est[:, :, 0, 1]
    W = wmax + 1
    d_idx, i_idx = np.meshgrid(np.arange(W), np.arange(N), indexing="ij")
    j_idx = np.minimum(i_idx + d_idx, N - 1)
    valid = ((i_idx + d_idx) <= N - 1)[None].astype(np.float64)
    with np.errstate(under="ignore"):
        ea = np.exp(np.minimum(t[:, i_idx, j_idx, :], 700.0)) * valid[..., None]
        eb = np.exp(np.minimum(t[:, j_idx, i_idx, :], 700.0)) * valid[..., None]
    # arc*go factors, [B, W, N] indexed [w, i]
    ea1 = ea[..., 1] * ego[:, :, 1, 1][:, i_idx]
    ea0 = ea[..., 0] * ego[:, :, 1, 0][:, i_idx]
    eb1 = eb[..., 1] * ego[:, :, 0, 1][:, j_idx]
    eb0 = eb[..., 0] * ego[:, :, 0, 0][:, j_idx]
    KR = np.zeros((B, W, N)); KL = np.zeros((B, W, N))
    IR = np.zeros((B, W, N)); IL = np.zeros((B, W, N))
    KR[:, 0] = 1.0
    KL[:, 0] = 1.0
    for w in range(1, W):
        s = N - w
        ir = np.zeros((B, s)); il = np.zeros((B, s))
        for tq in range(w):
            aR = (ea1 if tq > 0 else ea0)[:, w, :s]
            bL = (eb1 if tq < w - 1 else eb0)[:, w, :s]
            stopCL = sln[:, w:w + s] if tq == w - 1 else slh[:, w:w + s]
            stopCR = srn[:, :s] if tq == 0 else srh[:, :s]
            krkl = KR[:, tq, :s] * KL[:, w - 1 - tq, 1 + tq:1 + tq + s]
            ir += krkl * aR * stopCL
            il += krkl * bL * stopCR
        IR[:, w, :s] = ir
        IL[:, w, :s] = il
        kr = np.zeros((B, s)); kl = np.zeros((B, s))
        for tq in range(w):
            stop2 = srn[:, w:w + s] if tq == w - 1 else srh[:, 1 + tq:1 + tq + s]
            kr += IR[:, tq + 1, :s] * KR[:, w - 1 - tq, 1 + tq:1 + tq + s] * stop2
            stop3 = sln[:, :s] if tq == 0 else slh[:, tq:tq + s]
            kl += KL[:, tq, :s] * stop3 * IL[:, w - tq, tq:tq + s]
        KR[:, w, :s] = kr
        KL[:, w, :s] = kl
    ll = np.full((B, W), np.nan)
    with np.errstate(divide="ignore"):
        for L in range(1, W):
            ll[:, L] = np.log(KR[:, L, 0] * srh[:, 0])
    return ll


def make_in_maps(trans_scores, dec_scores):
    (PA1, PB1, PA0, PB0, PA3, PB3), (rhoR, rhoL), c0 = _host_tables(
        trans_scores, dec_scores)
    B = PA1.shape[0]
    coefs = np.stack([PA1, PB1, PA0, PB0, PA3, PB3], axis=2)  # [B,41,6,41]
    coefs = coefs.reshape(B, COEF_IN).astype(BFNP)
    stops = np.stack([rhoR, rhoL], axis=1).astype(BFNP)  # [B,2,41]
    sll = _short_ll(trans_scores, dec_scores)
    in_maps = []
    for c in range(NCORES):
        sl = slice(c * B_CORE, (c + 1) * B_CORE)
        # dram row = partition, packed [row][table][g][41] like SBUF
        cc = np.ascontiguousarray(
            coefs[sl].reshape(128, G, N, NTAB, N).transpose(0, 2, 3, 1, 4)
        ).reshape(128, G * COEF_IN)
        ss = np.ascontiguousarray(
            stops[sl].reshape(128, G, 2, N).transpose(0, 2, 1, 3)
        ).reshape(128, G * STOP_IN)
        in_maps.append({"coefs": cc, "stops": ss})
    return in_maps, (c0, sll)


def assemble(results, len_array, extra):
    c0, sll = extra
    ln = np.asarray(len_array).astype(np.int64)
    c0 = np.asarray(c0).astype(np.float64)
    B = len(ln)
    out = np.empty(B, dtype=np.float32)
    for c, res in enumerate(results):
        ecr = res["ecr"].reshape(B_CORE, N - 1).astype(np.float64)
        dsum = res["dsum"].reshape(B_CORE).astype(np.float64)
        lc = ln[c * B_CORE:(c + 1) * B_CORE]
        idx = np.arange(B_CORE)
        with np.errstate(divide="ignore"):
            out[c * B_CORE:(c + 1) * B_CORE] = (
                np.log(ecr[idx, lc - 1]) + dsum * np.log(2.0) * lc
                + c0[c * B_CORE:(c + 1) * B_CORE] * lc
            ).astype(np.float32)
    short = ln <= SHORT_LEN
    gl = np.arange(B)
    out[short] = sll[gl[short], ln[short]].astype(np.float32)
    return out


def kernel(trans_scores, dec_scores, len_array):
    from concourse.bass_utils import run_bass_kernel_spmd

    nc = get_nc()
    in_maps, extra = make_in_maps(trans_scores, dec_scores)
    res = run_bass_kernel_spmd(nc, in_maps, core_ids=list(range(NCORES)))
    return assemble(res.results, len_array, extra)


# revision 26
# speedup vs baseline: 1.2716x; 1.0042x over previous
"""DMV inside algorithm (Eisner chart DP, logsumexp semiring) on Trainium2.

Strategy (v2)
-------------
Pure data parallelism: 4096 sentences -> 8 cores x 512; per core ONE pass of
[128 SBUF partitions] x [G=4 sentence groups], all tables bf16 so every big
DVE tensor_tensor runs in 2x_1p mode (0.52 ns/elem).

Exp-domain DP with a positive-only boundary decomposition (no cancellation,
bf16-safe). Tables per group, diag-packed (row = span width, col = start):
  KR/KL: complete-without-stop, row 0 == 1
  SIR[r,c] = eIR[r,c] * srh[c+r]  (right-incomplete, child's R-stop folded)
  SIL[r,c] = eIL[r,c] * slh[c]    (left-incomplete,  child's L-stop folded)
Recurrences per width w (s = 41-w):
  S[i]   = sum_{t=1..w-2} KR[t,i] * KL[w-1-t, i+1+t]          (shared!)
  SIR[w] = PA1*S + PA0*KL[w-1,i+1] + PA3*KR[w-1,i]
  SIL[w] = PB1*S + PB0*KL[w-1,i+1] + PB3*KR[w-1,i]
  KR[w]  = sum_{t=0..w-2} SIR[t+1,i]*KR[w-1-t,i+1+t] + SIR[w,i]*rhoR[i+w]
  KL[w]  = sum_{t=0..w-2} KL[t+1,i]*SIL[w-1-t,i+1+t] + SIL[w,i]*rhoL[i]
with all six coefficient tables and rhoR=srn/srh, rhoL=sln/slh positive,
precomputed on host. Reductions are in-place binary trees of bf16 adds.
Renorm at w=20 rescales row d by an exact power of two 2^(-k*d) (exponent
trick), k returned per sentence (dsum) and undone on host.

Host covers len<=6 sentences with an exact f64 mini-DP (the 2e-2 relative
gate implies tiny absolute budgets only for very short sentences).
"""

import os

os.environ.setdefault("JAX_PLATFORMS", "cpu")

import numpy as np
import ml_dtypes

import concourse.bass as bass  # noqa: F401  (registers engine classes)
import concourse.tile as tile
import bass_rust
from concourse import bacc, mybir

F32 = mybir.dt.float32
BF16 = mybir.dt.bfloat16
OP = mybir.AluOpType
AX = mybir.AxisListType
BFNP = ml_dtypes.bfloat16

N = 41
D = 1681            # table pitch N*N
G = 4               # sentence groups per partition (1 pass = 512/core)
NCORES = 8
B_CORE = 128 * G
NTAB = 6            # coef tables: PA1, PB1, PA0, PB0, PA3, PB3
COEF_IN = NTAB * D
CROW = NTAB * G * N  # coefs are [row][table][g][41]: one width's coefficient
                     # read is a single compact interval, so the Tile checker
                     # links it to exactly one streaming DMA chunk
STOP_IN = 2 * N     # rhoR, rhoL
RENORM_W = 20
SHORT_LEN = 6       # host computes len <= SHORT_LEN exactly

# banks slots: 0..3 KR g0..3 | 4..7 SIL | 8..11 SIR | 12..15 KL
S_KR, S_SIL, S_SIR, S_KL = 0, 4, 8, 12

# bf16 scratch (element offsets)
PAP = 384           # per-group pitch of A-side product buffer (max (w-2)*s)
PBP = 424           # per-q pitch of B-side product buffer (max w*s)
ZB_PA = 0
ZB_PB = ZB_PA + 4 * PAP
ZB_U0 = ZB_PB + 8 * PBP
ZB_U3 = ZB_U0 + 2 * G * N
ZB_U03 = ZB_U3 + 2 * G * N
ZB_U1 = ZB_U03 + 2 * G * N
ZB_CORR = ZB_U1 + 2 * G * N
ZB_MB = ZB_CORR + 8 * N
ZB_MEXP = ZB_MB + G * 42
ZB_TOTAL = ZB_MEXP + G * 861

# f32 scratch
ZF_MU2 = 0          # [2,4]
ZF_MU = 8           # [4]
ZF_LM = 12          # [4]
ZF_SC = 16          # [4] 127-k
ZF_SCI = 20         # [4] bit-built 2^-k
ZF_DSUM = 24        # [4]
ZF_M = 28           # [4,42]
ZF_OUT = ZF_M + G * 42           # [4,40]
ZF_TOTAL = ZF_OUT + G * 40

def ap_of(t, offset, dims, lead=None):
    ap = t.copy()
    first = list(t.ap[0]) if lead is None else list(lead)
    ap.ap = bass_rust.VecI64Pair([first] + [list(d) for d in dims])
    ap.offset = offset
    return ap


def build_nc():
    nc = bacc.Bacc("TRN2", target_bir_lowering=False, debug=False, num_devices=1)
    coefs_in = nc.dram_tensor("coefs", [128, G * COEF_IN], BF16, kind="ExternalInput").ap()
    stops_in = nc.dram_tensor("stops", [128, G * STOP_IN], BF16, kind="ExternalInput").ap()
    logs_d = nc.dram_tensor("ecr", [B_CORE, N - 1], F32, kind="ExternalOutput").ap()
    dsum_d = nc.dram_tensor("dsum", [B_CORE], F32, kind="ExternalOutput").ap()

    with tile.TileContext(nc) as tc:
        with tc.tile_pool(name="p", bufs=1) as pool:
            banks_t = pool.tile([128, 16 * D], BF16)
            coefs_t = pool.tile([128, G * COEF_IN], BF16)
            stops_t = pool.tile([128, G * STOP_IN], BF16)
            zb_t = pool.tile([128, ZB_TOTAL], BF16)
            zf_t = pool.tile([128, ZF_TOTAL], F32)
            banks = banks_t[:]
            coefs = coefs_t[:]
            stops = stops_t[:]
            zb = zb_t[:]
            zf = zf_t[:]
            zi = zf.bitcast(mybir.dt.int32)

            v = nc.vector

            # ---- input DMA ----
            # coefs are row-interleaved ([g][row][6 tables][41]) so each
            # row-range chunk is one contiguous-per-group DMA with large
            # descriptors; chunks sized so arrival tracks DP consumption.
            chunks = ((1, 2), (2, 4), (4, 8), (8, 16), (16, 41))
            lo, hi = chunks[0]
            nc.sync.dma_start(
                ap_of(coefs, lo * CROW, [[1, (hi - lo) * CROW]]),
                ap_of(coefs_in, lo * CROW, [[1, (hi - lo) * CROW]],
                      lead=[G * COEF_IN, 128]),
            )
            nc.sync.dma_start(
                ap_of(stops, 0, [[1, G * STOP_IN]]),
                ap_of(stops_in, 0, [[1, G * STOP_IN]], lead=[G * STOP_IN, 128]),
            )
            for lo, hi in chunks[1:]:
                nc.sync.dma_start(
                    ap_of(coefs, lo * CROW, [[1, (hi - lo) * CROW]]),
                    ap_of(coefs_in, lo * CROW, [[1, (hi - lo) * CROW]],
                          lead=[G * COEF_IN, 128]),
                )

            # ---- init ----
            # The DP never writes: SIR/SIL row 0, and cols > 40-r of row r.
            # The renorm rescale reads full rows <= RENORM_W, so zero exactly
            # those cells (small Pool ops, disjoint from all DP writes, so
            # the DVE never waits on them).
            nc.gpsimd.memset(ap_of(banks, S_SIL * D, [[D, 8], [1, N]]), 0.0)
            for r in range(1, RENORM_W + 1):
                nc.gpsimd.memset(
                    ap_of(banks, r * N + (N - r), [[D, 16], [1, r]]), 0.0)
            v.memset(ap_of(zf, ZF_DSUM, [[1, G]]), 0.0)
            # KR[0,:] = KL[0,:] = 1
            v.memset(ap_of(banks, S_KR * D, [[12 * D, 2], [D, G], [1, N]]), 1.0)

            # ---- chart DP ----
            for w in range(1, N):
                s = N - w
                # B-side interior products first: they only need width w-1
                # tables, and give the Pool boundary ops a large window
                if w >= 2:
                    nb = w - 1
                    v.tensor_tensor(
                        ap_of(zb, ZB_PB, [[PBP, 8], [s, nb], [1, s]]),
                        ap_of(banks, S_SIR * D + N, [[D, 8], [N, nb], [1, s]]),
                        ap_of(banks, (w - 1) * N + 1, [[D, 8], [-(N - 1), nb], [1, s]]),
                        OP.mult,
                    )
                # A-side shared interior product + tree reduce
                if w >= 3:
                    na = w - 2
                    v.tensor_tensor(
                        ap_of(zb, ZB_PA, [[PAP, G], [s, na], [1, s]]),
                        ap_of(banks, S_KR * D + N, [[D, G], [N, na], [1, s]]),
                        ap_of(banks, S_KL * D + (w - 2) * N + 2,
                              [[D, G], [-(N - 1), na], [1, s]]),
                        OP.mult,
                    )
                    T = na
                    while T > 1:
                        h = T // 2
                        v.tensor_tensor(
                            ap_of(zb, ZB_PA, [[PAP, G], [s, h], [1, s]]),
                            ap_of(zb, ZB_PA, [[PAP, G], [s, h], [1, s]]),
                            ap_of(zb, ZB_PA + (T - h) * s,
                                  [[PAP, G], [s, h], [1, s]]),
                            OP.add,
                        )
                        T -= h
                # u0/u3 boundary terms; Pool engine once its fixed costs
                # hide under the DVE A-side product of the same width
                ub = nc.gpsimd if w >= 6 else v
                ub.tensor_tensor(
                    ap_of(zb, ZB_U0, [[G * N, 2], [N, G], [1, s]]),
                    ap_of(banks, S_KL * D + (w - 1) * N + 1,
                          [[0, 2], [D, G], [1, s]]),
                    ap_of(coefs, w * CROW + 2 * G * N,
                          [[G * N, 2], [N, G], [1, s]]),
                    OP.mult,
                )
                ub.tensor_tensor(
                    ap_of(zb, ZB_U3, [[G * N, 2], [N, G], [1, s]]),
                    ap_of(banks, S_KR * D + (w - 1) * N,
                          [[0, 2], [D, G], [1, s]]),
                    ap_of(coefs, w * CROW + 4 * G * N,
                          [[G * N, 2], [N, G], [1, s]]),
                    OP.mult,
                )
                wr_ap = ap_of(banks, S_SIR * D + w * N,
                              [[-4 * D, 2], [D, G], [1, s]])
                if w >= 3:
                    # u1 first: it only needs the DVE-side tree, so the
                    # wait for the Pool boundary ops overlaps its execution
                    v.tensor_tensor(
                        ap_of(zb, ZB_U1, [[G * N, 2], [N, G], [1, s]]),
                        ap_of(zb, ZB_PA, [[0, 2], [PAP, G], [1, s]]),
                        ap_of(coefs, w * CROW,
                              [[G * N, 2], [N, G], [1, s]]),
                        OP.mult,
                    )
                    v.tensor_tensor(
                        ap_of(zb, ZB_U03, [[G * N, 2], [N, G], [1, s]]),
                        ap_of(zb, ZB_U0, [[G * N, 2], [N, G], [1, s]]),
                        ap_of(zb, ZB_U3, [[G * N, 2], [N, G], [1, s]]),
                        OP.add,
                    )
                    # SIR[w]/SIL[w] = u03 + u1
                    v.tensor_tensor(
                        wr_ap,
                        ap_of(zb, ZB_U03, [[G * N, 2], [N, G], [1, s]]),
                        ap_of(zb, ZB_U1, [[G * N, 2], [N, G], [1, s]]),
                        OP.add,
                    )
                else:
                    # SIR[w]/SIL[w] = u0 + u3 (no interior)
                    v.tensor_tensor(
                        wr_ap,
                        ap_of(zb, ZB_U0, [[G * N, 2], [N, G], [1, s]]),
                        ap_of(zb, ZB_U3, [[G * N, 2], [N, G], [1, s]]),
                        OP.add,
                    )
                # B side: correction slab (t = w-1) then interior products
                kout_ap = ap_of(banks, S_KR * D + w * N,
                                [[12 * D, 2], [D, G], [1, s]])
                corr_out = (
                    kout_ap if w == 1 else
                    ap_of(zb, ZB_CORR, [[G * N, 2], [N, G], [1, s]])
                )
                cb = nc.gpsimd if w >= 6 else v
                cb.tensor_tensor(
                    corr_out,
                    ap_of(banks, S_SIR * D + w * N, [[-4 * D, 2], [D, G], [1, s]]),
                    ap_of(stops, w, [[G * N - w, 2], [N, G], [1, s]]),
                    OP.mult,
                )
                if w >= 2:
                    T = w - 1
                    while T > 1:
                        h = T // 2
                        v.tensor_tensor(
                            ap_of(zb, ZB_PB, [[PBP, 8], [s, h], [1, s]]),
                            ap_of(zb, ZB_PB, [[PBP, 8], [s, h], [1, s]]),
                            ap_of(zb, ZB_PB + (T - h) * s, [[PBP, 8], [s, h], [1, s]]),
                            OP.add,
                        )
                        T -= h
                    v.tensor_tensor(
                        kout_ap,
                        ap_of(zb, ZB_PB, [[4 * PBP, 2], [PBP, G], [1, s]]),
                        ap_of(zb, ZB_CORR, [[G * N, 2], [N, G], [1, s]]),
                        OP.add,
                    )

                if w == RENORM_W:
                    s0 = N - w
                    # mu[g] = max over KR/KL row w
                    v.reduce_max(
                        ap_of(zf, ZF_MU2, [[G, 2], [1, G]]),
                        ap_of(banks, S_KR * D + w * N, [[12 * D, 2], [D, G], [1, s0]]),
                        axis=AX.X,
                    )
                    v.tensor_tensor(
                        ap_of(zf, ZF_MU, [[1, G]]),
                        ap_of(zf, ZF_MU2, [[1, G]]),
                        ap_of(zf, ZF_MU2 + G, [[1, G]]),
                        OP.max,
                    )
                    # k = round(log2(mu)/w) via exponent bits: the float
                    # bit pattern X of mu gives log2(mu) ~= X/2^23 - 127
                    # (max err 0.086, absorbed by the rounding)
                    v.tensor_copy(
                        ap_of(zf, ZF_LM, [[1, G]]),
                        ap_of(zi, ZF_MU, [[1, G]]))
                    v.tensor_scalar(
                        ap_of(zf, ZF_LM, [[1, G]]), ap_of(zf, ZF_LM, [[1, G]]),
                        1.0 / (w * 2.0 ** 23), -127.0 / w,
                        OP.mult, OP.add,
                    )
                    v.tensor_scalar(
                        ap_of(zf, ZF_LM, [[1, G]]), ap_of(zf, ZF_LM, [[1, G]]),
                        12582912.0, 12582912.0, OP.add, OP.subtract,
                    )
                    v.tensor_tensor(
                        ap_of(zf, ZF_DSUM, [[1, G]]),
                        ap_of(zf, ZF_DSUM, [[1, G]]),
                        ap_of(zf, ZF_LM, [[1, G]]),
                        OP.add,
                    )
                    # 2^-k via exponent bits: (127 - k) << 23
                    v.tensor_scalar(
                        ap_of(zf, ZF_SC, [[1, G]]), ap_of(zf, ZF_LM, [[1, G]]),
                        -1.0, 127.0, OP.mult, OP.add,
                    )
                    v.tensor_copy(
                        ap_of(zi, ZF_SCI, [[1, G]]), ap_of(zf, ZF_SC, [[1, G]]))
                    v.tensor_scalar(
                        ap_of(zi, ZF_SCI, [[1, G]]), ap_of(zi, ZF_SCI, [[1, G]]),
                        23, None, OP.arith_shift_left,
                    )
                    # M[g,d] = 2^(-k d), multiplicative scan; then bf16 copy
                    v.memset(ap_of(zf, ZF_M, [[42, G], [1, 1]]), 1.0)
                    for g in range(G):
                        sca = ap_of(zf, ZF_SCI + g, [[0, N]])
                        v.tensor_tensor_scan(
                            ap_of(zf, ZF_M + g * 42 + 1, [[1, N]]),
                            sca, sca, 1.0, OP.mult, OP.bypass,
                        )
                    # M expanded to per-element rows (bf16, exact pow2),
                    # so the big rescale multiplies run in 2x_1p mode;
                    # expansion split across DVE/Pool to halve its latency
                    HB = 11 * N
                    v.tensor_copy(
                        ap_of(zb, ZB_MEXP, [[861, G], [1, HB]]),
                        ap_of(zf, ZF_M, [[42, G], [1, 11], [0, N]]))
                    nc.gpsimd.tensor_copy(
                        ap_of(zb, ZB_MEXP + HB, [[861, G], [1, (w + 1) * N - HB]]),
                        ap_of(zf, ZF_M + 11, [[42, G], [1, w - 10], [0, N]]))
                    tap = ap_of(banks, 0,
                                [[4 * D, 4], [D, G], [1, (w + 1) * N]])
                    v.tensor_tensor(
                        tap, tap,
                        ap_of(zb, ZB_MEXP, [[0, 4], [861, G], [1, (w + 1) * N]]),
                        OP.mult,
                    )
                    # coef rows w+1..40: one extra arc factor 2^-k per group
                    for g in range(G):
                        cap = ap_of(coefs, (w + 1) * CROW + g * N,
                                    [[CROW, N - 1 - w], [G * N, NTAB], [1, N]])
                        v.tensor_scalar(
                            cap, cap,
                            ap_of(zf, ZF_M + g * 42 + 1, [[1, 1]]), None,
                            OP.mult,
                        )
                    # rows <= RENORM_W of KR are final now: ship them early
                    v.tensor_copy(
                        ap_of(zf, ZF_OUT, [[N - 1, G], [1, RENORM_W]]),
                        ap_of(banks, S_KR * D + N, [[D, G], [N, RENORM_W]]),
                    )
                    nc.sync.dma_start(
                        ap_of(logs_d, 0, [[N - 1, G], [1, RENORM_W]],
                              lead=[G * (N - 1), 128]),
                        ap_of(zf, ZF_OUT, [[N - 1, G], [1, RENORM_W]]),
                    )
                    # dsum is final after the (single) renorm: ship it now
                    nc.sync.dma_start(
                        ap_of(dsum_d, 0, [[1, G]], lead=[G, 128]),
                        ap_of(zf, ZF_DSUM, [[1, G]]),
                    )

            # ---- output: KR col 0 rows 21..40 (rows 1..20 shipped at
            # renorm time) ----
            v.tensor_copy(
                ap_of(zf, ZF_OUT + RENORM_W, [[N - 1, G], [1, N - 1 - RENORM_W]]),
                ap_of(banks, S_KR * D + (RENORM_W + 1) * N,
                      [[D, G], [N, N - 1 - RENORM_W]]),
            )
            nc.sync.dma_start(
                ap_of(logs_d, RENORM_W, [[N - 1, G], [1, N - 1 - RENORM_W]],
                      lead=[G * (N - 1), 128]),
                ap_of(zf, ZF_OUT + RENORM_W, [[N - 1, G], [1, N - 1 - RENORM_W]]),
            )

    nc.compile()
    return nc


_NC_CACHE = {}


def get_nc():
    if "nc" not in _NC_CACHE:
        _NC_CACHE["nc"] = build_nc()
    return _NC_CACHE["nc"]


def _host_tables(trans_scores, dec_scores):
    """f32 coefficient tables (diag-packed [B, d, i]) + rho vectors + c0."""
    t = np.asarray(trans_scores, dtype=np.float32)
    dec = np.asarray(dec_scores, dtype=np.float32)
    B = t.shape[0]
    go = dec[..., 0]
    st = dec[..., 1]
    tm = np.where(t < -1e8, -np.inf, t).max(axis=3)
    with np.errstate(invalid="ignore"):
        colmax = tm.max(axis=1)
        proxy = np.nanmean(
            np.where(np.isfinite(colmax), colmax, np.nan)[:, 1:], axis=-1)
    c0 = np.clip(np.nan_to_num(proxy + 0.5), -20.0, 20.0).astype(np.float32)
    with np.errstate(under="ignore"):
        E = np.exp(t - c0[:, None, None, None])
        ego = np.exp(go)
        est = np.exp(st)
    d_idx, i_idx = np.meshgrid(np.arange(N), np.arange(N), indexing="ij")
    j_idx = np.minimum(i_idx + d_idx, N - 1)
    valid = ((i_idx + d_idx) <= N - 1)[None].astype(np.float32)
    ea = E[:, i_idx, j_idx, :]
    eb = E[:, j_idx, i_idx, :]
    a1 = ea[..., 1] * ego[:, :, 1, 1][:, i_idx] * valid
    a0 = ea[..., 0] * ego[:, :, 1, 0][:, i_idx] * valid
    b1 = eb[..., 1] * ego[:, :, 0, 1][:, j_idx] * valid
    b0 = eb[..., 0] * ego[:, :, 0, 0][:, j_idx] * valid
    srn, srh = est[:, :, 1, 0], est[:, :, 1, 1]
    sln, slh = est[:, :, 0, 0], est[:, :, 0, 1]
    slh_j = slh[:, j_idx] * valid
    sln_j = sln[:, j_idx] * valid
    srh_j = srh[:, j_idx] * valid
    srh_i, srn_i, slh_i = srh[:, i_idx], srn[:, i_idx], slh[:, i_idx]
    PA1 = a1 * slh_j * srh_j
    PA0 = a0 * slh_j * srh_j
    PA3 = a1 * sln_j * srh_j
    PB1 = b1 * srh_i * slh_i * valid
    PB0 = b1 * srn_i * slh_i * valid
    PB3 = b0 * srh_i * slh_i * valid
    PA0[:, 1] = a0[:, 1] * sln_j[:, 1] * srh_j[:, 1]
    PA1[:, 1] = 0.0
    PA3[:, 1] = 0.0
    PB0[:, 1] = b0[:, 1] * srn_i[:, 1] * slh_i[:, 1]
    PB1[:, 1] = 0.0
    PB3[:, 1] = 0.0
    rhoR = srn / srh
    rhoL = sln / slh
    return (PA1, PB1, PA0, PB0, PA3, PB3), (rhoR, rhoL), c0


def _short_ll(trans_scores, dec_scores, wmax=SHORT_LEN):
    """Exact f64 LL for len <= wmax: truncated exp-domain DP, direct
    reference recurrences (diag-packed [B, row=width, col=start])."""
    t = np.asarray(trans_scores, dtype=np.float64)
    dec = np.asarray(dec_scores, dtype=np.float64)
    B = t.shape[0]
    ego, est = np.exp(dec[..., 0]), np.exp(dec[..., 1])
    srn, srh = est[:, :, 1, 0], est[:, :, 1, 1]
    sln, slh = est[:, :, 0, 0], est[:, :, 0, 1]
    W = wmax + 1
    d_idx, i_idx = np.meshgrid(np.arange(W), np.arange(N), indexing="ij")
    j_idx = np.minimum(i_idx + d_idx, N - 1)
    valid = ((i_idx + d_idx) <= N - 1)[None].astype(np.float64)
    with np.errstate(under="ignore"):
        ea = np.exp(np.minimum(t[:, i_idx, j_idx, :], 700.0)) * valid[..., None]
        eb = np.exp(np.minimum(t[:, j_idx, i_idx, :], 700.0)) * valid[..., None]
    # arc*go factors, [B, W, N] indexed [w, i]
    ea1 = ea[..., 1] * ego[:, :, 1, 1][:, i_idx]
    ea0 = ea[..., 0] * ego[:, :, 1, 0][:, i_idx]
    eb1 = eb[..., 1] * ego[:, :, 0, 1][:, j_idx]
    eb0 = eb[..., 0] * ego[:, :, 0, 0][:, j_idx]
    KR = np.zeros((B, W, N)); KL = np.zeros((B, W, N))
    IR = np.zeros((B, W, N)); IL = np.zeros((B, W, N))
    KR[:, 0] = 1.0
    KL[:, 0] = 1.0
    for w in range(1, W):
        s = N - w
        ir = np.zeros((B, s)); il = np.zeros((B, s))
        for tq in range(w):
            aR = (ea1 if tq > 0 else ea0)[:, w, :s]
            bL = (eb1 if tq < w - 1 else eb0)[:, w, :s]
            stopCL = sln[:, w:w + s] if tq == w - 1 else slh[:, w:w + s]
            stopCR = srn[:, :s] if tq == 0 else srh[:, :s]
            krkl = KR[:, tq, :s] * KL[:, w - 1 - tq, 1 + tq:1 + tq + s]
            ir += krkl * aR * stopCL
            il += krkl * bL * stopCR
        IR[:, w, :s] = ir
        IL[:, w, :s] = il
        kr = np.zeros((B, s)); kl = np.zeros((B, s))
        for tq in range(w):
            stop2 = srn[:, w:w + s] if tq == w - 1 else srh[:, 1 + tq:1 + tq + s]
            kr += IR[:, tq + 1, :s] * KR[:, w - 1 - tq, 1 + tq:1 + tq + s] * stop2
            stop3 = sln[:, :s] if tq == 0 else slh[:, tq:tq + s]
            kl += KL[:, tq, :s] * stop3 * IL[:, w - tq, tq:tq + s]
        KR[:, w, :s] = kr
        KL[:, w, :s] = kl
    ll = np.full((B, W), np.nan)
    with np.errstate(divide="ignore"):
        for L in range(1, W):
            ll[:, L] = np.log(KR[:, L, 0] * srh[:, 0])
    return ll


def make_in_maps(trans_scores, dec_scores):
    (PA1, PB1, PA0, PB0, PA3, PB3), (rhoR, rhoL), c0 = _host_tables(
        trans_scores, dec_scores)
    B = PA1.shape[0]
    coefs = np.stack([PA1, PB1, PA0, PB0, PA3, PB3], axis=2)  # [B,41,6,41]
    coefs = coefs.reshape(B, COEF_IN).astype(BFNP)
    stops = np.stack([rhoR, rhoL], axis=1).astype(BFNP)  # [B,2,41]
    sll = _short_ll(trans_scores, dec_scores)
    in_maps = []
    for c in range(NCORES):
        sl = slice(c * B_CORE, (c + 1) * B_CORE)
        # dram row = partition, packed [row][table][g][41] like SBUF
        cc = np.ascontiguousarray(
            coefs[sl].reshape(128, G, N, NTAB, N).transpose(0, 2, 3, 1, 4)
        ).reshape(128, G * COEF_IN)
        ss = np.ascontiguousarray(
            stops[sl].reshape(128, G, 2, N).transpose(0, 2, 1, 3)
        ).reshape(128, G * STOP_IN)
        in_maps.append({"coefs": cc, "stops": ss})
    return in_maps, (c0, sll)


def assemble(results, len_array, extra):
    c0, sll = extra
    ln = np.asarray(len_array).astype(np.int64)
    c0 = np.asarray(c0).astype(np.float64)
    B = len(ln)
    out = np.empty(B, dtype=np.float32)
    for c, res in enumerate(results):
        ecr = res["ecr"].reshape(B_CORE, N - 1).astype(np.float64)
        dsum = res["dsum"].reshape(B_CORE).astype(np.float64)
        lc = ln[c * B_CORE:(c + 1) * B_CORE]
        idx = np.arange(B_CORE)
        with np.errstate(divide="ignore"):
            out[c * B_CORE:(c + 1) * B_CORE] = (
                np.log(ecr[idx, lc - 1]) + dsum * np.log(2.0) * lc
                + c0[c * B_CORE:(c + 1) * B_CORE] * lc
            ).astype(np.float32)
    short = ln <= SHORT_LEN
    gl = np.arange(B)
    out[short] = sll[gl[short], ln[short]].astype(np.float32)
    return out


def kernel(trans_scores, dec_scores, len_array):
    from concourse.bass_utils import run_bass_kernel_spmd

    nc = get_nc()
    in_maps, extra = make_in_maps(trans_scores, dec_scores)
    res = run_bass_kernel_spmd(nc, in_maps, core_ids=list(range(NCORES)))
    return assemble(res.results, len_array, extra)
